# revision 1
# baseline (speedup 1.0000x reference)
"""Trainium2 Bass kernel for nn_ActorCritic (LSTM with done-resets + heads).

Sharding: TIME-sharded. The done-resets (p=0.5/step) make state older than
~30 steps irrelevant, so each core processes a K-step span: a warmup from
zero state re-synchronizes (h,c) exactly (every env is guaranteed a reset
inside the warmup window by construction), then the owned steps. K and the
7 segment boundaries are computed AT RUNTIME from the actual done data
(binary search for the smallest uniform span such that greedy boundary
placement covers T=512), so the result is exact for any input. Each core
sees the FULL batch B=256; no collectives.

Host-side marshalling (not compute): x is cast to bf16 and pre-transposed to
x^T_aug [294, K*256] with row 292 = 1.0 (folds gate bias into the xW GEMM)
and row 293 = done_t scaled by -30 into the f-gate column block (sigmoid(f)
-> 0 on reset steps, which zeroes the c-history exactly like the reference's
c*(1-d) mask). Gate blocks are reordered [o,i,f,g] and the g block (weights
+ bias) is pre-doubled so one sigmoid yields sigma(2g), with
tanh(g) = 2*sigma(2g)-1 recovered inside the fused DVE tail.

Device per core, per step (B=256 as two interleaved 128-wide half-batches so
the two serial recurrence chains hide each other's latency):
  - xW GEMM (3 K-tiles x 4 gates, 256-col pieces covering both halves)
    streams ~2 steps ahead directly into the step's [128,1024] PSUM tile
    (2 banks; one start=True per 2KB zero region); W_hh matmuls accumulate
    on top. No SBUF xw staging, no fold matmul, no PSUM->SBUF copies.
  - ACT per half: sigmoid over [i,f,g] (critical path), sigmoid over [o]
    (off-path), tanh(c_new).
  - DVE per half (bf16, 2x mode): t2=sig_f*c; u=(sig_g'-0.5)*sig_i;
    c_new=2u+t2 (== sig_f*c + sig_i*tanh(g)); hm=som*tanh(c).
  - Pool per half (off the critical chain): som=sig_o*m,
    h=sig_o*tanh(c) into the bf16 history consumed by the heads.
  - Heads ride at the top of each step: 2 matmuls (16 cols) + fused
    bias-add/copy on DVE + one DMA per step into a padded [K*256,16]
    output (host strips the pad).
"""

import sys
from contextlib import ExitStack

import numpy as np

sys.path.insert(0, "/opt/trn_rl_repo")

# Problem constants (hardcoded per harness contract).
T = 512
B = 256
NCORES = 8
IN = 292
H = 128
A = 12
NOUT = 13
HB = 128  # half-batch width

INA = IN + 2  # +ones row (bias), +done row (f-gate kill)
KSPLITS = [(0, 128), (128, 128), (256, INA - 256)]
MAXTCH = 26  # max steps per input chunk (SBUF budget)


def _chunks(K):
    nch = -(-K // MAXTCH)
    base = K // nch
    rem = K - base * nch
    return [base + (1 if i < rem else 0) for i in range(nch)]


def build_nc(K):
    import concourse.bass as bass
    import concourse.tile as tile
    from concourse import bacc, mybir

    f32 = mybir.dt.float32
    bf16 = mybir.dt.bfloat16
    AF = mybir.ActivationFunctionType
    OP = mybir.AluOpType

    tchs = _chunks(K)
    NCH = len(tchs)
    coff = [0]
    for tc_ in tchs:
        coff.append(coff[-1] + tc_)
    step_chunk = []
    for ch, tc_ in enumerate(tchs):
        step_chunk += [ch] * tc_

    nc = bacc.Bacc("TRN2", target_bir_lowering=False, debug=False)

    # ---- I/O (all per-core slices prepared by host) ----
    xt_d = nc.dram_tensor("xt", [INA, K * B], bf16, kind="ExternalInput").ap()
    m_d = nc.dram_tensor("m", [128, K * B], bf16, kind="ExternalInput").ap()
    h0_d = nc.dram_tensor("h0", [128, B], bf16, kind="ExternalInput").ap()
    c0_d = nc.dram_tensor("c0", [128, B], bf16, kind="ExternalInput").ap()
    wih_d = nc.dram_tensor("wih", [INA, 512], bf16, kind="ExternalInput").ap()
    whh_d = nc.dram_tensor("whh", [128, 512], bf16, kind="ExternalInput").ap()
    wcat_d = nc.dram_tensor("wcat", [128, 16], bf16, kind="ExternalInput").ap()
    bhd_d = nc.dram_tensor("bhd", [128, 32], f32, kind="ExternalInput").ap()
    out_d = nc.dram_tensor("out", [K * B, 16], f32, kind="ExternalOutput").ap()

    with tile.TileContext(nc) as tc, ExitStack() as ctx:
        cst = ctx.enter_context(tc.tile_pool(name="cst", bufs=1))
        big = ctx.enter_context(tc.tile_pool(name="big", bufs=1))
        xtp = ctx.enter_context(tc.tile_pool(name="xtp", bufs=2))
        mp = ctx.enter_context(tc.tile_pool(name="mp", bufs=2))
        wk = ctx.enter_context(tc.tile_pool(name="wk", bufs=3))
        pg_pool = ctx.enter_context(tc.tile_pool(name="pg", bufs=3, space="PSUM"))
        php = ctx.enter_context(tc.tile_pool(name="ph", bufs=2, space="PSUM"))

        # ---- persistent tiles ----
        wih_sb = [cst.tile([sz, 512], bf16, tag=f"wih{k}", name=f"wih{k}")
                  for k, (_, sz) in enumerate(KSPLITS)]
        whh_sb = cst.tile([128, 512], bf16, tag="whh", name="whh")
        wcat_sb = cst.tile([128, 16], bf16, tag="wcat", name="wcat")
        bhd_sb = cst.tile([128, 32], f32, tag="bhd", name="bhd")
        h0_sb = cst.tile([128, B], bf16, tag="h0", name="h0")
        c0_sb = cst.tile([128, B], bf16, tag="c0", name="c0")
        hs_all = big.tile([128, K * B], bf16, tag="hs", name="hs")

        for k, (off, sz) in enumerate(KSPLITS):
            nc.sync.dma_start(out=wih_sb[k][:, :], in_=wih_d[off:off + sz, :])
        nc.sync.dma_start(out=whh_sb[:, :], in_=whh_d[:, :])
        nc.sync.dma_start(out=wcat_sb[:, :], in_=wcat_d[:, :])
        nc.sync.dma_start(out=bhd_sb[:, :], in_=bhd_d[:, :])
        nc.sync.dma_start(out=h0_sb[:, :], in_=h0_d[:, :])
        nc.sync.dma_start(out=c0_sb[:, :], in_=c0_d[:, :])

        # ---- input chunk DMAs ----
        xts = {}
        mts = {}

        def load_chunk(ch):
            if ch >= NCH:
                return
            cols = slice(coff[ch] * B, coff[ch + 1] * B)
            n = tchs[ch] * B
            tiles = []
            for k, (off, sz) in enumerate(KSPLITS):
                xt = xtp.tile([sz, MAXTCH * B], bf16, tag=f"xt{k}", name=f"xt{k}")
                nc.sync.dma_start(out=xt[:, 0:n], in_=xt_d[off:off + sz, cols])
                tiles.append(xt)
            xts[ch] = tiles
            mt = mp.tile([128, MAXTCH * B], bf16, tag="mt", name="mt")
            nc.sync.dma_start(out=mt[:, 0:n], in_=m_d[:, cols])
            mts[ch] = mt

        load_chunk(0)
        load_chunk(1)

        # ---- xW GEMM straight into the step's PSUM tile ----
        # One [128, 4*256] tile per step (2 banks; cols = slot*256 + b).
        # Slot order [o, i, f, g]. Each piece covers BOTH halves (256 moving
        # cols per stationary load). ONE start per 2KB zero region (slot 0 /
        # slot 2 first k-piece); every address's first write in the group
        # auto-zeroes, so later slots accumulate correctly.
        psum_tiles = {}
        PIECES = [(slot, k) for slot in range(4) for k in range(len(KSPLITS))]

        def emit_xw(t, pieces):
            if t >= K:
                return
            if t in psum_tiles:
                pg = psum_tiles[t]
            else:
                pg = pg_pool.tile([128, 1024], f32, tag="pg", name="pg")
                psum_tiles[t] = pg
            tiles = xts[step_chunk[t]]
            c0_ = (t - coff[step_chunk[t]]) * B
            for slot, k in pieces:
                off, sz = KSPLITS[k]
                nc.tensor.matmul(
                    pg[:, slot * B:slot * B + B],
                    wih_sb[k][0:sz, slot * 128:(slot + 1) * 128],
                    tiles[k][0:sz, c0_:c0_ + B],
                    start=(slot in (0, 2) and k == 0), stop=False)

        emit_xw(0, PIECES)
        emit_xw(1, PIECES)

        hm_prev = [h0_sb[:, 0:HB], h0_sb[:, HB:B]]
        c_prev = [c0_sb[:, 0:HB], c0_sb[:, HB:B]]

        def emit_heads(t):
            ph = php.tile([128, 512], f32, tag="ph", name="ph")
            for hb in range(2):
                nc.tensor.matmul(ph[:, hb * 16:hb * 16 + 16],
                                 hs_all[:, t * B + hb * HB:t * B + hb * HB + HB],
                                 wcat_sb[:, :], start=(hb == 0), stop=(hb == 1))
            ob = wk.tile([128, 32], f32, tag="ob", name="ob")
            nc.vector.scalar_tensor_tensor(
                ob[:, :], ph[:, 0:32], 1.0, bhd_sb[:, :], OP.mult, OP.add)
            nc.sync.dma_start(
                out=out_d[t * B:(t + 1) * B, :].rearrange(
                    "(a p) s -> p a s", a=2, p=128),
                in_=ob[:, :].rearrange("p (a s) -> p a s", a=2))

        # ---- the recurrence ----
        # Slot order [o, i, f, g]: sigma over slots 1:4 ([i,f,g]) is the only
        # ACT op on the critical path; sigma(o) runs off-path for som/hs.
        for t in range(K):
            if t > 0 and t - 1 in coff:
                load_chunk(coff.index(t - 1) + 2)
            mt = mts[step_chunk[t]]
            mc0 = (t - coff[step_chunk[t]]) * B
            if t > 1:
                emit_heads(t - 2)  # 2 steps of slack so hs is never waited on
            pg = psum_tiles.pop(t)
            pgv = pg[:, :].rearrange("p (s h b) -> p s h b", s=4, h=2, b=HB)

            sig = [None, None]
            for hb in range(2):
                for slot in range(4):
                    nc.tensor.matmul(
                        pg[:, slot * B + hb * HB:slot * B + hb * HB + HB],
                        whh_sb[:, slot * 128:(slot + 1) * 128],
                        hm_prev[hb], start=False,
                        stop=(hb == 1 and slot in (1, 3)))
                # xW fillers split so whh(h1) sits early in the PE queue
                # (only ~2 pieces behind whh(h0)), keeping the h1 chain's
                # phase offset small while the wait still has PE cover.
                emit_xw(t + 2, PIECES[0:2] if hb == 0 else PIECES[2:12])
                s = wk.tile([128, 512], bf16, tag=f"sig{hb}", name=f"sig{hb}")
                nc.scalar.activation(
                    s[:, 128:512].rearrange("p (s b) -> p s b", s=3),
                    pgv[:, 1:4, hb, :], AF.Sigmoid)
                sig[hb] = s

            # tail (per half): t2 = sig_f*c; u = (sig_g' - 0.5)*sig_i;
            # c_new = 2u + t2  (== sig_f*c + sig_i*(2*sig(2g)-1))
            cn = [None, None]
            for hb in range(2):
                t2 = wk.tile([128, HB], bf16, tag=f"t2{hb}", name=f"t2{hb}")
                nc.vector.tensor_mul(t2[:, :], sig[hb][:, 256:384], c_prev[hb])
                u = wk.tile([128, HB], bf16, tag=f"u{hb}", name=f"u{hb}")
                nc.vector.scalar_tensor_tensor(
                    u[:, :], sig[hb][:, 384:512], 0.5, sig[hb][:, 128:256],
                    OP.subtract, OP.mult)
                c_new = wk.tile([128, HB], bf16, tag=f"cn{hb}", name=f"cn{hb}")
                nc.vector.scalar_tensor_tensor(
                    c_new[:, :], u[:, :], 2.0, t2[:, :], OP.mult, OP.add)
                cn[hb] = c_new
                # off-path sigma(o) right after the chain ops are queued
                nc.scalar.activation(sig[hb][:, 0:128], pgv[:, 0, hb, :],
                                     AF.Sigmoid)

            # Pool queue gets both som's BEFORE the (slack) hs writes so
            # neither half's hm stalls behind the other's history write.
            thc = [None, None]
            for hb in range(2):
                th = wk.tile([128, HB], bf16, tag=f"th{hb}", name=f"th{hb}")
                nc.scalar.activation(th[:, :], cn[hb][:, :], AF.Tanh)
                thc[hb] = th
                if t < K - 1:
                    # som = sig_o*m on DVE, off the critical chain (runs
                    # during tanh); hm = som*tanh(c) closes the chain. Kept
                    # off Pool so the scheduler's cost model (Pool sems are
                    # ~1.4us there) doesn't predict hm late and push the
                    # next whh far back in the static PE queue.
                    som = wk.tile([128, HB], bf16, tag=f"som{hb}",
                                  name=f"som{hb}")
                    nc.vector.tensor_mul(
                        som[:, :], sig[hb][:, 0:128],
                        mt[:, mc0 + hb * HB:mc0 + hb * HB + HB])
                    hm = wk.tile([128, HB], bf16, tag=f"hm{hb}", name=f"hm{hb}")
                    nc.vector.tensor_mul(hm[:, :], som[:, :], th[:, :])
                    hm_prev[hb] = hm
                c_prev[hb] = cn[hb]
            for hb in range(2):
                # h into the bf16 history (heads-only consumer)
                col = t * B + hb * HB
                nc.vector.tensor_mul(hs_all[:, col:col + HB],
                                     sig[hb][:, 0:128], thc[hb][:, :])
        emit_heads(K - 2)
        emit_heads(K - 1)

    nc.compile()
    return nc


_NC = {}


def _get_nc(K):
    if K not in _NC:
        _NC[K] = build_nc(K)
    return _NC[K]


def _segments(done):
    """Smallest uniform span K and greedy owned ranges [(t_own0, t_own1)]
    such that every env has a reset inside each warmup window."""
    last = np.full(B, -10**9, dtype=np.int64)
    last_min = np.zeros(T, dtype=np.int64)
    for t in range(T):
        last = np.where(done[t] == 1, t, last)
        last_min[t] = last.min()
    Wt = np.arange(T) - last_min  # lookback needed at owned-start t

    def plan(K):
        end = min(K, T)
        segs = [(0, end)]
        for _ in range(1, NCORES):
            if end >= T:
                break
            t_c = end
            cap = K - Wt[t_c]
            if cap <= 0:
                return None
            end = min(t_c + cap, T)
            segs.append((t_c, end))
        if end < T:
            return None
        while len(segs) < NCORES:  # degenerate: fewer segments needed
            segs.append((T, T))
        return segs

    lo, hi = 8, T
    while lo < hi:
        mid = (lo + hi) // 2
        if plan(mid) is not None:
            hi = mid
        else:
            lo = mid + 1
    return lo, plan(lo)


def _make_in_maps(inputs, K, segs):
    import ml_dtypes

    bf16 = ml_dtypes.bfloat16
    x = np.asarray(inputs["x"], dtype=np.float32)
    done = np.asarray(inputs["done"], dtype=np.int32)
    h0 = np.asarray(inputs["h0"], dtype=np.float32).reshape(B, H)
    c0 = np.asarray(inputs["c0"], dtype=np.float32).reshape(B, H)
    Wih = np.asarray(inputs["W_ih"], dtype=np.float32)
    Whh = np.asarray(inputs["W_hh"], dtype=np.float32)
    bias = (np.asarray(inputs["b_ih"], dtype=np.float32)
            + np.asarray(inputs["b_hh"], dtype=np.float32)).reshape(4 * H)
    Wpi = np.asarray(inputs["W_pi"], dtype=np.float32)
    bpi = np.asarray(inputs["b_pi"], dtype=np.float32).reshape(A)
    Wv = np.asarray(inputs["W_v"], dtype=np.float32)
    bv = np.asarray(inputs["b_v"], dtype=np.float32).reshape(1)

    # gate order i,f,g,o -> o,i,f,g; g block (weights + bias) pre-doubled
    order = np.r_[384:512, 0:128, 128:256, 256:384]
    GS = 384  # g block offset after reorder
    FS = 256  # f block offset after reorder
    WihR = Wih[order].copy()
    WihR[GS:GS + 128] *= 2.0
    WhhR = Whh[order].copy()
    WhhR[GS:GS + 128] *= 2.0
    biasR = bias[order].copy()
    biasR[GS:GS + 128] *= 2.0

    wih_aug = np.zeros((INA, 512), dtype=np.float32)
    wih_aug[0:IN] = WihR.T
    wih_aug[IN] = biasR
    wih_aug[IN + 1, FS:FS + 128] = -30.0  # done kills the f gate
    wih_bf = wih_aug.astype(bf16)
    whh_bf = np.ascontiguousarray(WhhR.T).astype(bf16)

    wcat = np.zeros((128, 16), dtype=np.float32)
    wcat[:, 0:A] = Wpi.T
    wcat[:, A] = Wv[0]
    wcat_bf = wcat.astype(bf16)
    bhd = np.zeros((128, 32), dtype=np.float32)
    for hb in range(2):
        bhd[:, hb * 16:hb * 16 + A] = bpi
        bhd[:, hb * 16 + A] = bv[0]

    in_maps = []
    for c in range(NCORES):
        t0 = max(segs[c][1] - K, 0)  # span start (warmup-padded)
        xseg = x[t0:t0 + K]
        dseg = done[t0:t0 + K].astype(np.float32)
        xt = np.empty((INA, K * B), dtype=np.float32)
        xt[0:IN] = xseg.transpose(2, 0, 1).reshape(IN, K * B)
        xt[IN] = 1.0
        xt[IN + 1] = dseg.reshape(K * B)

        m = np.ones((K, B), dtype=np.float32)
        m[0:K - 1] = 1.0 - dseg[1:K]
        m_bc = np.ascontiguousarray(
            np.broadcast_to(m.reshape(1, K * B), (128, K * B))).astype(bf16)

        if t0 == 0:
            h0c = (h0.T * (1.0 - dseg[0])[None, :]).astype(bf16)
            c0c = np.ascontiguousarray(c0.T).astype(bf16)
        else:
            h0c = np.zeros((H, B), dtype=bf16)
            c0c = np.zeros((H, B), dtype=bf16)

        in_maps.append({
            "xt": xt.astype(bf16),
            "m": m_bc,
            "h0": np.ascontiguousarray(h0c),
            "c0": c0c,
            "wih": wih_bf,
            "whh": whh_bf,
            "wcat": wcat_bf,
            "bhd": bhd,
        })
    return in_maps


def _try_device_reset():
    try:
        import ctypes

        import jax

        jax.devices()
        lib = ctypes.CDLL("/opt/axon/libaxon_pjrt.so")
        if hasattr(lib, "axon_reset"):
            lib.axon_reset.restype = ctypes.c_int64
            lib.axon_reset()
    except Exception:
        pass


def kernel(**inputs):
    from concourse.bass_utils import run_bass_kernel_spmd

    done = np.asarray(inputs["done"], dtype=np.int32)
    K, segs = _segments(done)
    nc = _get_nc(K)
    in_maps = _make_in_maps(inputs, K, segs)
    try:
        res = run_bass_kernel_spmd(nc, in_maps, core_ids=list(range(NCORES)))
    except Exception:
        _try_device_reset()
        res = run_bass_kernel_spmd(nc, in_maps, core_ids=list(range(NCORES)))
    outs = [r["out"].reshape(K, B, 16)[:, :, 0:NOUT] for r in res.results]
    full = np.empty((T, B, NOUT), dtype=np.float32)
    for c in range(NCORES):
        o0, o1 = segs[c]
        if o1 <= o0:
            continue
        t0 = max(o1 - K, 0)
        full[o0:o1] = outs[c][o0 - t0:o1 - t0]
    return full.reshape(T * B, NOUT).copy()



# revision 4
# speedup vs baseline: 1.9792x; 1.9792x over previous
"""Trainium2 Bass kernel for nn_ActorCritic (LSTM with done-resets + heads).

Segment-packed formulation. done ~ Bernoulli(0.5) per (t, env) resets (h, c)
at the START of step t, so the T=512 scan factorizes into ~T*B/2 independent
segments (mean length 2, max ~18). Host-side we split every env's timeline
into segments, deal them across the 8 cores (LPT by positions), and bin-pack
each core's segments into WIDTH=768 column chains of depth L (~22). This is
EXACT for any input (no warmup redundancy): resets inside a chain are handled
by the baseline's -30*done f-gate kill (c history) and the m mask (h history),
and chain position 0 gets (h0, c0) for columns seeded with an env's initial
segment. Serial depth drops 72 -> ~22 while every macro-step stays 768 wide.

Device per core, per macro-step: 3 independent 256-col chunks (global chunk
index g = 3*s + j; the recurrence chain is g -> g+3, so each engine always
has ~2 chunks of unrelated work to hide the chain latency):
  - xW GEMM (3 K-tiles x 4 gates, 256-col pieces) streams 2 chunks ahead
    into the chunk's [128,1024] PSUM tile (2 banks, one start per bank);
    4 W_hh matmuls (N=256) accumulate on top.
  - ONE sigmoid over the whole [128,1024] gate tile (slot order [o,i,f,g],
    g block pre-doubled so tanh(g) = 2*sig(2g)-1), ONE tanh per chunk.
  - DVE tail (bf16 2x): t2 = sig_f*c; u = (sig_g'-0.5)*sig_i; c = 2u+t2;
    hs = sig_o*tanh(c) into the bf16 history; hm = hs*m for the recurrence.
  - Heads: 2 matmuls per chunk accumulate into a [128,512] PSUM bank shared
    by 16 chunks; one fused bias-add + one DMA per 16 chunks.

Host-side marshalling (not compute): segment packing, x gather into packed
column order (bf16, +ones row for bias, +done row for the f-kill), m mask
broadcast, output scatter back to (t, env) order.
"""

import heapq
import sys
from contextlib import ExitStack

import numpy as np

sys.path.insert(0, "/opt/trn_rl_repo")

# Problem constants (hardcoded per harness contract).
T = 512
B = 256
NCORES = 8
IN = 292
H = 128
A = 12
NOUT = 13

INA = IN + 2  # +ones row (bias), +done row (f-gate kill)
KSPLITS = [(0, 128), (128, 128), (256, INA - 256)]
CH = 256  # chunk width (cols)
NJ = 3  # chunks per macro-step
WIDTH = NJ * CH  # column chains per core
HGRP = 16  # chunks per shared heads PSUM bank
GCH = 24  # g-chunks per input slab (SBUF budget)


def _chunks(NG):
    nch = -(-NG // GCH)
    base = NG // nch
    rem = NG - base * nch
    return [base + (1 if i < rem else 0) for i in range(nch)]


def build_nc(L):
    import concourse.bass as bass
    import concourse.tile as tile
    from concourse import bacc, mybir

    f32 = mybir.dt.float32
    bf16 = mybir.dt.bfloat16
    AF = mybir.ActivationFunctionType
    OP = mybir.AluOpType

    NG = NJ * L
    gchs = _chunks(NG)
    NCH = len(gchs)
    coff = [0]
    for n in gchs:
        coff.append(coff[-1] + n)
    g_slab = []
    for ch, n in enumerate(gchs):
        g_slab += [ch] * n

    nc = bacc.Bacc("TRN2", target_bir_lowering=False, debug=False)

    # ---- I/O (all per-core slices prepared by host) ----
    xt_d = nc.dram_tensor("xt", [INA, NG * CH], bf16, kind="ExternalInput").ap()
    m_d = nc.dram_tensor("m", [128, NG * CH], bf16, kind="ExternalInput").ap()
    h0_d = nc.dram_tensor("h0", [128, WIDTH], bf16, kind="ExternalInput").ap()
    c0_d = nc.dram_tensor("c0", [128, WIDTH], bf16, kind="ExternalInput").ap()
    wih_d = nc.dram_tensor("wih", [INA, 512], bf16, kind="ExternalInput").ap()
    whh_d = nc.dram_tensor("whh", [128, 512], bf16, kind="ExternalInput").ap()
    wcat_d = nc.dram_tensor("wcat", [128, 16], bf16, kind="ExternalInput").ap()
    bhd_d = nc.dram_tensor("bhd", [128, 512], f32, kind="ExternalInput").ap()
    out_d = nc.dram_tensor("out", [NG * CH, 16], f32, kind="ExternalOutput").ap()

    with tile.TileContext(nc) as tc, ExitStack() as ctx:
        cst = ctx.enter_context(tc.tile_pool(name="cst", bufs=1))
        big = ctx.enter_context(tc.tile_pool(name="big", bufs=1))
        xtp = ctx.enter_context(tc.tile_pool(name="xtp", bufs=2))
        mp = ctx.enter_context(tc.tile_pool(name="mp", bufs=2))
        wk = ctx.enter_context(tc.tile_pool(name="wk", bufs=3))
        pg_pool = ctx.enter_context(tc.tile_pool(name="pg", bufs=3, space="PSUM"))
        php = ctx.enter_context(tc.tile_pool(name="ph", bufs=2, space="PSUM"))

        # ---- persistent tiles ----
        wih_sb = [cst.tile([sz, 512], bf16, tag=f"wih{k}", name=f"wih{k}")
                  for k, (_, sz) in enumerate(KSPLITS)]
        whh_sb = cst.tile([128, 512], bf16, tag="whh", name="whh")
        wcat_sb = cst.tile([128, 16], bf16, tag="wcat", name="wcat")
        bhd_sb = cst.tile([128, 512], f32, tag="bhd", name="bhd")
        h0_sb = cst.tile([128, WIDTH], bf16, tag="h0", name="h0")
        c0_sb = cst.tile([128, WIDTH], bf16, tag="c0", name="c0")
        hs_all = big.tile([128, NG * CH], bf16, tag="hs", name="hs")

        for k, (off, sz) in enumerate(KSPLITS):
            nc.sync.dma_start(out=wih_sb[k][:, :], in_=wih_d[off:off + sz, :])
        nc.sync.dma_start(out=whh_sb[:, :], in_=whh_d[:, :])
        nc.sync.dma_start(out=wcat_sb[:, :], in_=wcat_d[:, :])
        nc.sync.dma_start(out=bhd_sb[:, :], in_=bhd_d[:, :])
        nc.sync.dma_start(out=h0_sb[:, :], in_=h0_d[:, :])
        nc.sync.dma_start(out=c0_sb[:, :], in_=c0_d[:, :])

        # ---- input slab DMAs ----
        xts = {}
        mts = {}

        def load_slab(ch):
            if ch >= NCH:
                return
            cols = slice(coff[ch] * CH, coff[ch + 1] * CH)
            n = gchs[ch] * CH
            tiles = []
            for k, (off, sz) in enumerate(KSPLITS):
                xt = xtp.tile([sz, GCH * CH], bf16, tag=f"xt{k}", name=f"xt{k}")
                nc.sync.dma_start(out=xt[:, 0:n], in_=xt_d[off:off + sz, cols])
                tiles.append(xt)
            xts[ch] = tiles
            mt = mp.tile([128, GCH * CH], bf16, tag="mt", name="mt")
            nc.sync.dma_start(out=mt[:, 0:n], in_=m_d[:, cols])
            mts[ch] = mt

        load_slab(0)
        load_slab(1)

        # ---- xW GEMM straight into the chunk's PSUM tile ----
        # One [128, 4*256] tile per chunk (2 banks; cols = slot*256 + c).
        # Slot order [o, i, f, g]. ONE start per 2KB bank (slot 0 / slot 2
        # first k-piece); later slots' first writes auto-zero, accumulation
        # is per-element via has_written.
        psum_tiles = {}
        PIECES = [(slot, k) for slot in range(4) for k in range(len(KSPLITS))]

        def emit_xw(g):
            if g >= NG:
                return
            pg = pg_pool.tile([128, 1024], f32, tag="pg", name="pg")
            psum_tiles[g] = pg
            tiles = xts[g_slab[g]]
            c0_ = (g - coff[g_slab[g]]) * CH
            for slot, k in PIECES:
                off, sz = KSPLITS[k]
                nc.tensor.matmul(
                    pg[:, slot * CH:slot * CH + CH],
                    wih_sb[k][0:sz, slot * 128:(slot + 1) * 128],
                    tiles[k][0:sz, c0_:c0_ + CH],
                    start=(slot in (0, 2) and k == 0), stop=False)

        # heads: chunk g's 2 matmuls land in shared bank ph[g // HGRP] at
        # cols (g % HGRP)*32; one accumulation group per bank.
        ph_tiles = {}

        def emit_heads(g):
            grp, loc = g // HGRP, g % HGRP
            if loc == 0:
                ph_tiles[grp] = php.tile([128, 512], f32, tag="ph", name="ph")
            ph = ph_tiles[grp]
            last_in_grp = (loc == HGRP - 1) or (g == NG - 1)
            for hb in range(2):
                nc.tensor.matmul(ph[:, loc * 32 + hb * 16:loc * 32 + hb * 16 + 16],
                                 hs_all[:, g * CH + hb * 128:g * CH + hb * 128 + 128],
                                 wcat_sb[:, :], start=(loc == 0 and hb == 0),
                                 stop=(last_in_grp and hb == 1))

        def flush_heads(grp, n):
            # bias add + one DMA for the n chunks of group grp
            ph = ph_tiles.pop(grp)
            ob = wk.tile([128, 512], f32, tag="ob", name="ob")
            nc.vector.scalar_tensor_tensor(
                ob[:, 0:n * 32], ph[:, 0:n * 32], 1.0, bhd_sb[:, 0:n * 32],
                OP.mult, OP.add)
            r0 = grp * HGRP * CH
            nc.sync.dma_start(
                out=out_d[r0:r0 + n * 2 * 128, :].rearrange(
                    "(c a p) s -> p c a s", a=2, p=128),
                in_=ob[:, 0:n * 32].rearrange("p (c a s) -> p c a s", a=2, s=16))

        emit_xw(0)
        emit_xw(1)

        hm_ref = {}
        c_ref = {}
        for j in range(NJ):
            hm_ref[j - NJ] = h0_sb[:, j * CH:(j + 1) * CH]
            c_ref[j - NJ] = c0_sb[:, j * CH:(j + 1) * CH]

        sig_ref = {}
        th_ref = {}

        # ---- the recurrence over global chunks ----
        for g in range(NG):
            if g > 0 and g - 1 in coff:
                load_slab(coff.index(g - 1) + 2)
            mt = mts[g_slab[g]]
            mc = (g - coff[g_slab[g]]) * CH

            # PE: W_hh accumulation for chunk g (chain input from g-3)
            pg = psum_tiles.pop(g)
            hm_in = hm_ref.pop(g - NJ)
            for slot in range(4):
                nc.tensor.matmul(
                    pg[:, slot * CH:slot * CH + CH],
                    whh_sb[:, slot * 128:(slot + 1) * 128],
                    hm_in, start=False, stop=(slot in (1, 3)))

            # PE fillers: xW two chunks ahead, heads two chunks behind
            emit_xw(g + 2)
            if g >= 2:
                emit_heads(g - 2)
                if (g - 2) % HGRP == HGRP - 1:
                    flush_heads((g - 2) // HGRP, HGRP)

            # ACT: tanh of last chunk's c (input was ready long ago), then
            # the one big sigmoid for this chunk.
            if g >= 1:
                th = wk.tile([128, CH], bf16, tag="th", name="th")
                nc.scalar.activation(th[:, :], c_ref[g - 1][:, :], AF.Tanh)
                th_ref[g - 1] = th
            sig = wk.tile([128, 1024], bf16, tag="sig", name="sig")
            nc.scalar.activation(sig[:, :], pg[:, :], AF.Sigmoid)
            sig_ref[g] = sig

            # DVE: finish chunk g-1 (hs into history, masked hm for the
            # chain), then chunk g's c-path.
            if g >= 1:
                sp = sig_ref[g - 1]
                col = (g - 1) * CH
                nc.vector.tensor_mul(hs_all[:, col:col + CH],
                                     sp[:, 0:CH], th_ref.pop(g - 1)[:, :])
                if (g - 1) // NJ < L - 1:
                    hm = wk.tile([128, CH], bf16, tag="hm", name="hm")
                    nc.vector.tensor_mul(hm[:, :], hs_all[:, col:col + CH],
                                         mts[g_slab[g - 1]][
                                             :, (g - 1 - coff[g_slab[g - 1]])
                                             * CH:(g - coff[g_slab[g - 1]]) * CH])
                    hm_ref[g - 1] = hm
                del sig_ref[g - 1]
            t2 = wk.tile([128, CH], bf16, tag="t2", name="t2")
            nc.vector.tensor_mul(t2[:, :], sig[:, 2 * CH:3 * CH],
                                 c_ref.pop(g - NJ)[:, :])
            u = wk.tile([128, CH], bf16, tag="u", name="u")
            nc.vector.scalar_tensor_tensor(
                u[:, :], sig[:, 3 * CH:4 * CH], 0.5, sig[:, CH:2 * CH],
                OP.subtract, OP.mult)
            cn = wk.tile([128, CH], bf16, tag="cn", name="cn")
            nc.vector.scalar_tensor_tensor(
                cn[:, :], u[:, :], 2.0, t2[:, :], OP.mult, OP.add)
            c_ref[g] = cn

        # ---- drain: last chunk's tanh/hs, remaining heads ----
        th = wk.tile([128, CH], bf16, tag="th", name="th")
        nc.scalar.activation(th[:, :], c_ref[NG - 1][:, :], AF.Tanh)
        nc.vector.tensor_mul(hs_all[:, (NG - 1) * CH:NG * CH],
                             sig_ref[NG - 1][:, 0:CH], th[:, :])
        for g in (NG - 2, NG - 1):
            emit_heads(g)
            if g % HGRP == HGRP - 1:
                flush_heads(g // HGRP, HGRP)
        if (NG - 1) % HGRP != HGRP - 1:
            flush_heads((NG - 1) // HGRP, (NG - 1) % HGRP + 1)

    nc.compile()
    return nc


_NC = {}


def _get_nc(L):
    if L not in _NC:
        _NC[L] = build_nc(L)
    return _NC[L]


def _segments(done):
    """Split every env's timeline at done=1 into segments, deal segments
    across cores, pack each core's segments into WIDTH column chains.
    Returns (L, plan): L = max chain depth; plan[c] = per-core slot maps."""
    done = np.asarray(done, dtype=np.int32)
    segs = []  # (length, t0, b, initial)
    for b in range(B):
        col = done[:, b]
        starts = np.flatnonzero(col == 1)
        if len(starts) == 0 or starts[0] != 0:
            starts = np.r_[0, starts]
        lens = np.diff(np.r_[starts, T])
        for t0, ln in zip(starts.tolist(), lens.tolist()):
            segs.append((int(ln), int(t0), b, t0 == 0 and col[0] == 0))

    segs.sort(key=lambda s: (-s[0], s[1], s[2]))
    init_segs = [s for s in segs if s[3]]
    rest_segs = [s for s in segs if not s[3]]

    # deal across cores: initial segments round-robin, rest LPT by positions
    core_segs = [[] for _ in range(NCORES)]
    core_load = [0] * NCORES
    for i, s in enumerate(init_segs):
        c = i % NCORES
        core_segs[c].append(s)
        core_load[c] += s[0]
    heap = [(core_load[c], c) for c in range(NCORES)]
    heapq.heapify(heap)
    for s in rest_segs:
        load, c = heapq.heappop(heap)
        core_segs[c].append(s)
        heapq.heappush(heap, (load + s[0], c))

    # pack each core's segments into WIDTH columns (initial segs first, one
    # per column at position 0; then LPT over all columns)
    plan = []
    Lmax = 0
    for c in range(NCORES):
        ini = [s for s in core_segs[c] if s[3]]
        oth = [s for s in core_segs[c] if not s[3]]
        oth.sort(key=lambda s: (-s[0], s[1], s[2]))
        cols = [[] for _ in range(WIDTH)]
        fill = [0] * WIDTH
        for i, s in enumerate(ini):
            cols[i].append(s)
            fill[i] = s[0]
        heap = [(fill[w], w) for w in range(WIDTH)]
        heapq.heapify(heap)
        for s in oth:
            f, w = heapq.heappop(heap)
            cols[w].append(s)
            heapq.heappush(heap, (f + s[0], w))
        Lc = max(sum(s[0] for s in cols[w]) for w in range(WIDTH))
        Lmax = max(Lmax, Lc)
        plan.append({"cols": cols, "n_init": len(ini)})
    L = Lmax

    # build slot maps
    for c in range(NCORES):
        cols = plan[c]["cols"]
        src = np.full((L, WIDTH), -1, dtype=np.int64)
        de = np.ones((L, WIDTH), dtype=np.float32)  # done row (pads -> 1)
        m = np.zeros((L, WIDTH), dtype=np.float32)
        h0b = np.full(WIDTH, -1, dtype=np.int64)  # env idx for init state
        for w in range(WIDTH):
            s_off = 0
            for (ln, t0, b, initial) in cols[w]:
                ts = np.arange(t0, t0 + ln)
                src[s_off:s_off + ln, w] = ts * B + b
                de[s_off, w] = 0.0 if initial else 1.0
                de[s_off + 1:s_off + ln, w] = 0.0
                m[s_off:s_off + ln - 1, w] = 1.0
                if initial:
                    h0b[w] = b
                s_off += ln
        plan[c] = {"src": src, "de": de, "m": m, "h0b": h0b}
    return L, plan


def _make_in_maps(inputs, L, plan):
    import ml_dtypes

    bf16 = ml_dtypes.bfloat16
    x = np.asarray(inputs["x"], dtype=np.float32)
    done = np.asarray(inputs["done"], dtype=np.int32)
    h0 = np.asarray(inputs["h0"], dtype=np.float32).reshape(B, H)
    c0 = np.asarray(inputs["c0"], dtype=np.float32).reshape(B, H)
    Wih = np.asarray(inputs["W_ih"], dtype=np.float32)
    Whh = np.asarray(inputs["W_hh"], dtype=np.float32)
    bias = (np.asarray(inputs["b_ih"], dtype=np.float32)
            + np.asarray(inputs["b_hh"], dtype=np.float32)).reshape(4 * H)
    Wpi = np.asarray(inputs["W_pi"], dtype=np.float32)
    bpi = np.asarray(inputs["b_pi"], dtype=np.float32).reshape(A)
    Wv = np.asarray(inputs["W_v"], dtype=np.float32)
    bv = np.asarray(inputs["b_v"], dtype=np.float32).reshape(1)

    # gate order i,f,g,o -> o,i,f,g; g block (weights + bias) pre-doubled
    order = np.r_[384:512, 0:128, 128:256, 256:384]
    GS = 384  # g block offset after reorder
    FS = 256  # f block offset after reorder
    WihR = Wih[order].copy()
    WihR[GS:GS + 128] *= 2.0
    WhhR = Whh[order].copy()
    WhhR[GS:GS + 128] *= 2.0
    biasR = bias[order].copy()
    biasR[GS:GS + 128] *= 2.0

    wih_aug = np.zeros((INA, 512), dtype=np.float32)
    wih_aug[0:IN] = WihR.T
    wih_aug[IN] = biasR
    wih_aug[IN + 1, FS:FS + 128] = -30.0  # done kills the f gate
    wih_bf = wih_aug.astype(bf16)
    whh_bf = np.ascontiguousarray(WhhR.T).astype(bf16)

    wcat = np.zeros((128, 16), dtype=np.float32)
    wcat[:, 0:A] = Wpi.T
    wcat[:, A] = Wv[0]
    wcat_bf = wcat.astype(bf16)
    bgrp = np.zeros(16, dtype=np.float32)
    bgrp[0:A] = bpi
    bgrp[A] = bv[0]
    bhd = np.tile(bgrp, 32)[None, :].repeat(128, axis=0).copy()  # [128, 512]

    xT = np.ascontiguousarray(x.transpose(2, 0, 1).reshape(IN, T * B))
    h0T = h0.T  # [H, B]
    c0T = c0.T

    in_maps = []
    for c in range(NCORES):
        p = plan[c]
        src = p["src"].reshape(-1)  # [L*WIDTH] in slot order
        valid = src >= 0
        xt = np.zeros((INA, L * WIDTH), dtype=np.float32)
        xt[0:IN, valid] = xT[:, src[valid]]
        xt[IN] = 1.0
        xt[IN + 1] = p["de"].reshape(-1)

        m_bc = np.ascontiguousarray(np.broadcast_to(
            p["m"].reshape(1, L * WIDTH), (128, L * WIDTH))).astype(bf16)

        h0c = np.zeros((H, WIDTH), dtype=np.float32)
        c0c = np.zeros((H, WIDTH), dtype=np.float32)
        wsel = p["h0b"] >= 0
        h0c[:, wsel] = h0T[:, p["h0b"][wsel]]
        c0c[:, wsel] = c0T[:, p["h0b"][wsel]]

        in_maps.append({
            "xt": xt.astype(bf16),
            "m": m_bc,
            "h0": h0c.astype(bf16),
            "c0": c0c.astype(bf16),
            "wih": wih_bf,
            "whh": whh_bf,
            "wcat": wcat_bf,
            "bhd": bhd,
        })
    return in_maps


def _try_device_reset():
    try:
        import ctypes

        import jax

        jax.devices()
        lib = ctypes.CDLL("/opt/axon/libaxon_pjrt.so")
        if hasattr(lib, "axon_reset"):
            lib.axon_reset.restype = ctypes.c_int64
            lib.axon_reset()
    except Exception:
        pass


def kernel(**inputs):
    from concourse.bass_utils import run_bass_kernel_spmd

    done = np.asarray(inputs["done"], dtype=np.int32)
    L, plan = _segments(done)
    nc = _get_nc(L)
    in_maps = _make_in_maps(inputs, L, plan)
    try:
        res = run_bass_kernel_spmd(nc, in_maps, core_ids=list(range(NCORES)))
    except Exception:
        _try_device_reset()
        res = run_bass_kernel_spmd(nc, in_maps, core_ids=list(range(NCORES)))
    full = np.empty((T * B, NOUT), dtype=np.float32)
    for c in range(NCORES):
        out = res.results[c]["out"]  # [NG*CH, 16] in slot order
        src = plan[c]["src"].reshape(-1)
        valid = src >= 0
        full[src[valid]] = out[valid][:, 0:NOUT]
    return full


# revision 8
# speedup vs baseline: 2.1755x; 1.0991x over previous
"""Trainium2 Bass kernel for nn_ActorCritic (LSTM with done-resets + heads).

Segment-packed formulation. done ~ Bernoulli(0.5) per (t, env) resets (h, c)
at the START of step t, so the T=512 scan factorizes into ~T*B/2 independent
segments (mean length 2, max ~18). Host-side we split every env's timeline
into segments, deal them across the 8 cores (LPT by positions), and bin-pack
each core's segments into WIDTH=768 column chains of depth L (~22). This is
EXACT for any input (no warmup redundancy): resets inside a chain are handled
by the baseline's -30*done f-gate kill (c history) and the m mask (h history),
and chain position 0 gets (h0, c0) for columns seeded with an env's initial
segment. Serial depth drops 72 -> ~22 while every macro-step stays 768 wide.

Device per core, per macro-step: 3 independent 256-col chunks (global chunk
index g = 3*s + j; the recurrence chain is g -> g+3, so each engine always
has ~2 chunks of unrelated work to hide the chain latency):
  - xW GEMM (3 K-tiles x 4 gates, 256-col pieces) streams 2 chunks ahead
    into the chunk's [128,1024] PSUM tile (2 banks, one start per bank);
    4 W_hh matmuls (N=256) accumulate on top.
  - ONE sigmoid over the whole [128,1024] gate tile (slot order [o,i,f,g],
    g block pre-doubled so tanh(g) = 2*sig(2g)-1), ONE tanh per chunk.
  - DVE tail (bf16 2x): t2 = sig_f*c; u = (sig_g'-0.5)*sig_i; c = 2u+t2;
    hs = sig_o*tanh(c) into the bf16 history; hm = hs*m for the recurrence.
  - Heads: 2 matmuls per chunk accumulate into a [128,512] PSUM bank shared
    by 16 chunks; one fused bias-add + one DMA per 16 chunks.

Host-side marshalling (not compute): segment packing, x gather into packed
column order (bf16, +ones row for bias, +done row for the f-kill), m mask
broadcast, output scatter back to (t, env) order.
"""

import heapq
import sys
from contextlib import ExitStack

import numpy as np

sys.path.insert(0, "/opt/trn_rl_repo")

# Problem constants (hardcoded per harness contract).
T = 512
B = 256
NCORES = 8
IN = 292
H = 128
A = 12
NOUT = 13

INA = IN + 2  # +ones row (bias), +done row (f-gate kill)
KSPLITS = [(0, 128), (128, 128), (256, INA - 256)]
CH = 256  # chunk width (cols)
NJ = 3  # chunks per macro-step
WIDTH = NJ * CH  # column chains per core
HGRP = 16  # chunks per shared heads PSUM bank
GCH = 24  # max g-chunks per input slab (SBUF budget)


def _chunks(NG):
    # small first slab so the first xW can start ~8us earlier
    sizes = [8]
    left = NG - 8
    while left > 0:
        n = min(GCH, left)
        sizes.append(n)
        left -= n
    return sizes


def build_nc(L):
    import concourse.bass as bass
    import concourse.tile as tile
    from concourse import bacc, mybir

    f32 = mybir.dt.float32
    bf16 = mybir.dt.bfloat16
    AF = mybir.ActivationFunctionType
    OP = mybir.AluOpType

    NG = NJ * L
    gchs = _chunks(NG)
    NCH = len(gchs)
    coff = [0]
    for n in gchs:
        coff.append(coff[-1] + n)
    g_slab = []
    for ch, n in enumerate(gchs):
        g_slab += [ch] * n

    nc = bacc.Bacc("TRN2", target_bir_lowering=False, debug=False)

    # ---- I/O (all per-core slices prepared by host) ----
    xt_d = nc.dram_tensor("xt", [INA, NG * CH], bf16, kind="ExternalInput").ap()
    m_d = nc.dram_tensor("m", [128, NG * CH], bf16, kind="ExternalInput").ap()
    h0_d = nc.dram_tensor("h0", [128, WIDTH], bf16, kind="ExternalInput").ap()
    c0_d = nc.dram_tensor("c0", [128, WIDTH], bf16, kind="ExternalInput").ap()
    wih_d = nc.dram_tensor("wih", [INA, 512], bf16, kind="ExternalInput").ap()
    whh_d = nc.dram_tensor("whh", [128, 512], bf16, kind="ExternalInput").ap()
    wcat_d = nc.dram_tensor("wcat", [128, 16], bf16, kind="ExternalInput").ap()
    bhd_d = nc.dram_tensor("bhd", [128, 512], f32, kind="ExternalInput").ap()
    out_d = nc.dram_tensor("out", [NG * CH, 16], f32, kind="ExternalOutput").ap()

    with tile.TileContext(nc) as tc, ExitStack() as ctx:
        cst = ctx.enter_context(tc.tile_pool(name="cst", bufs=1))
        big = ctx.enter_context(tc.tile_pool(name="big", bufs=1))
        xtp = ctx.enter_context(tc.tile_pool(name="xtp", bufs=2))
        mp = ctx.enter_context(tc.tile_pool(name="mp", bufs=2))
        wk = ctx.enter_context(tc.tile_pool(name="wk", bufs=3))
        pg_pool = ctx.enter_context(tc.tile_pool(name="pg", bufs=3, space="PSUM"))
        php = ctx.enter_context(tc.tile_pool(name="ph", bufs=2, space="PSUM"))

        # ---- persistent tiles ----
        wih_sb = [cst.tile([sz, 512], bf16, tag=f"wih{k}", name=f"wih{k}")
                  for k, (_, sz) in enumerate(KSPLITS)]
        whh_sb = cst.tile([128, 512], bf16, tag="whh", name="whh")
        wcat_sb = cst.tile([128, 16], bf16, tag="wcat", name="wcat")
        bhd_sb = cst.tile([128, 512], f32, tag="bhd", name="bhd")
        h0_sb = cst.tile([128, WIDTH], bf16, tag="h0", name="h0")
        c0_sb = cst.tile([128, WIDTH], bf16, tag="c0", name="c0")
        hs_all = big.tile([128, NG * CH], bf16, tag="hs", name="hs")

        for k, (off, sz) in enumerate(KSPLITS):
            nc.sync.dma_start(out=wih_sb[k][:, :], in_=wih_d[off:off + sz, :])
        nc.sync.dma_start(out=whh_sb[:, :], in_=whh_d[:, :])
        nc.sync.dma_start(out=wcat_sb[:, :], in_=wcat_d[:, :])
        nc.sync.dma_start(out=bhd_sb[:, :], in_=bhd_d[:, :])
        nc.sync.dma_start(out=h0_sb[:, :], in_=h0_d[:, :])
        nc.sync.dma_start(out=c0_sb[:, :], in_=c0_d[:, :])

        # ---- input slab DMAs ----
        xts = {}
        mts = {}

        def load_slab(ch):
            if ch >= NCH:
                return
            n = gchs[ch] * CH
            c0_, c1_ = coff[ch] * CH, coff[ch + 1] * CH
            h = n // 2  # split DMAs for queue parallelism
            tiles = []
            for k, (off, sz) in enumerate(KSPLITS):
                xt = xtp.tile([sz, GCH * CH], bf16, tag=f"xt{k}", name=f"xt{k}")
                nc.sync.dma_start(out=xt[:, 0:h],
                                  in_=xt_d[off:off + sz, c0_:c0_ + h])
                nc.sync.dma_start(out=xt[:, h:n],
                                  in_=xt_d[off:off + sz, c0_ + h:c1_])
                tiles.append(xt)
            xts[ch] = tiles
            mt = mp.tile([128, GCH * CH], bf16, tag="mt", name="mt")
            nc.sync.dma_start(out=mt[:, 0:h], in_=m_d[:, c0_:c0_ + h])
            nc.sync.dma_start(out=mt[:, h:n], in_=m_d[:, c0_ + h:c1_])
            mts[ch] = mt

        load_slab(0)
        load_slab(1)

        # ---- PE warmup + ACT table preload ----
        # HAM starts cold (1.2 GHz) and needs ~3.4us of sustained PE work to
        # un-throttle; dummy matmuls on the (small, early) whh tile warm it
        # while the first x slab DMA is in flight. The sigmoid table load
        # (~2.7us) is also hoisted off the first chunk's critical path.
        scr = php.tile([128, 512], f32, tag="ph", name="scr")
        for i in range(14):
            nc.tensor.matmul(scr[:, :], whh_sb[:, 0:128], whh_sb[0:128, :],
                             start=True, stop=False)
        dumm = wk.tile([128, 16], bf16, tag="dumm", name="dumm")
        nc.scalar.activation(dumm[:, :], h0_sb[:, 0:16], AF.Sigmoid)

        def fillers(k):
            for _ in range(k):
                nc.tensor.matmul(scr[:, :], whh_sb[:, 0:128],
                                 whh_sb[0:128, :], start=True, stop=False)

        # ---- xW GEMM straight into the chunk's PSUM tile ----
        # One [128, 4*256] tile per chunk (2 banks; cols = slot*256 + c).
        # Slot order [o, i, f, g]. ONE start per 2KB bank (slot 0 / slot 2
        # first k-piece); later slots' first writes auto-zero, accumulation
        # is per-element via has_written.
        psum_tiles = {}
        PIECES = [(slot, k) for slot in range(4) for k in range(len(KSPLITS))]

        def emit_xw(g):
            if g >= NG:
                return
            pg = pg_pool.tile([128, 1024], f32, tag="pg", name="pg")
            psum_tiles[g] = pg
            tiles = xts[g_slab[g]]
            c0_ = (g - coff[g_slab[g]]) * CH
            for slot, k in PIECES:
                off, sz = KSPLITS[k]
                nc.tensor.matmul(
                    pg[:, slot * CH:slot * CH + CH],
                    wih_sb[k][0:sz, slot * 128:(slot + 1) * 128],
                    tiles[k][0:sz, c0_:c0_ + CH],
                    start=(slot in (0, 2) and k == 0), stop=False)

        # heads: chunk g's 2 matmuls land in shared bank ph[g // HGRP] at
        # cols (g % HGRP)*32; one accumulation group per bank.
        ph_tiles = {}

        def emit_heads(g):
            grp, loc = g // HGRP, g % HGRP
            if loc == 0:
                ph_tiles[grp] = php.tile([128, 512], f32, tag="ph", name="ph")
            ph = ph_tiles[grp]
            last_in_grp = (loc == HGRP - 1) or (g == NG - 1)
            for hb in range(2):
                nc.tensor.matmul(ph[:, loc * 32 + hb * 16:loc * 32 + hb * 16 + 16],
                                 hs_all[:, g * CH + hb * 128:g * CH + hb * 128 + 128],
                                 wcat_sb[:, :], start=(loc == 0 and hb == 0),
                                 stop=(last_in_grp and hb == 1))

        def flush_heads(grp, n):
            # bias add + one DMA for the n chunks of group grp
            ph = ph_tiles.pop(grp)
            ob = wk.tile([128, 512], f32, tag="ob", name="ob")
            nc.vector.scalar_tensor_tensor(
                ob[:, 0:n * 32], ph[:, 0:n * 32], 1.0, bhd_sb[:, 0:n * 32],
                OP.mult, OP.add)
            r0 = grp * HGRP * CH
            nc.sync.dma_start(
                out=out_d[r0:r0 + n * 2 * 128, :].rearrange(
                    "(c a p) s -> p c a s", a=2, p=128),
                in_=ob[:, 0:n * 32].rearrange("p (c a s) -> p c a s", a=2, s=16))

        emit_xw(0)
        emit_xw(1)

        hm_ref = {}
        c_ref = {}
        for j in range(NJ):
            hm_ref[j - NJ] = h0_sb[:, j * CH:(j + 1) * CH]
            c_ref[j - NJ] = c0_sb[:, j * CH:(j + 1) * CH]

        sig_ref = {}
        th_ref = {}

        # ---- the recurrence over global chunks ----
        for g in range(NG):
            if g > 0 and g - 1 in coff:
                load_slab(coff.index(g - 1) + 2)
            mt = mts[g_slab[g]]
            mc = (g - coff[g_slab[g]]) * CH

            # PE: W_hh accumulation for chunk g (chain input from g-3)
            pg = psum_tiles.pop(g)
            hm_in = hm_ref.pop(g - NJ)
            for slot in range(4):
                nc.tensor.matmul(
                    pg[:, slot * CH:slot * CH + CH],
                    whh_sb[:, slot * 128:(slot + 1) * 128],
                    hm_in, start=False, stop=(slot in (1, 3)))

            # PE fillers: xW two chunks ahead, heads two chunks behind
            emit_xw(g + 2)
            if g < 9:
                # keep PE busy/warm through pipeline priming
                fillers(6 if g == 0 else 4 if g < 4 else 2)
            if g >= 2:
                emit_heads(g - 2)
                if (g - 2) % HGRP == HGRP - 1:
                    flush_heads((g - 2) // HGRP, HGRP)

            # ACT: tanh of last chunk's c (input was ready long ago), then
            # the one big sigmoid for this chunk.
            if g >= 1:
                th = wk.tile([128, CH], bf16, tag="th", name="th")
                nc.scalar.activation(th[:, :], c_ref[g - 1][:, :], AF.Tanh)
                th_ref[g - 1] = th
            sig = wk.tile([128, 1024], bf16, tag="sig", name="sig")
            nc.scalar.activation(sig[:, :], pg[:, :], AF.Sigmoid)
            sig_ref[g] = sig

            # DVE: finish chunk g-1 (hs into history, masked hm for the
            # chain), then chunk g's c-path.
            if g >= 1:
                sp = sig_ref[g - 1]
                col = (g - 1) * CH
                nc.vector.tensor_mul(hs_all[:, col:col + CH],
                                     sp[:, 0:CH], th_ref.pop(g - 1)[:, :])
                if (g - 1) // NJ < L - 1:
                    hm = wk.tile([128, CH], bf16, tag="hm", name="hm")
                    nc.vector.tensor_mul(hm[:, :], hs_all[:, col:col + CH],
                                         mts[g_slab[g - 1]][
                                             :, (g - 1 - coff[g_slab[g - 1]])
                                             * CH:(g - coff[g_slab[g - 1]]) * CH])
                    hm_ref[g - 1] = hm
                del sig_ref[g - 1]
            t2 = wk.tile([128, CH], bf16, tag="t2", name="t2")
            nc.vector.tensor_mul(t2[:, :], sig[:, 2 * CH:3 * CH],
                                 c_ref.pop(g - NJ)[:, :])
            u = wk.tile([128, CH], bf16, tag="u", name="u")
            nc.vector.scalar_tensor_tensor(
                u[:, :], sig[:, 3 * CH:4 * CH], 0.5, sig[:, CH:2 * CH],
                OP.subtract, OP.mult)
            cn = wk.tile([128, CH], bf16, tag="cn", name="cn")
            nc.vector.scalar_tensor_tensor(
                cn[:, :], u[:, :], 2.0, t2[:, :], OP.mult, OP.add)
            c_ref[g] = cn

        # ---- drain: last chunk's tanh/hs, remaining heads ----
        th = wk.tile([128, CH], bf16, tag="th", name="th")
        nc.scalar.activation(th[:, :], c_ref[NG - 1][:, :], AF.Tanh)
        nc.vector.tensor_mul(hs_all[:, (NG - 1) * CH:NG * CH],
                             sig_ref[NG - 1][:, 0:CH], th[:, :])
        for g in (NG - 2, NG - 1):
            emit_heads(g)
            if g % HGRP == HGRP - 1:
                flush_heads(g // HGRP, HGRP)
        if (NG - 1) % HGRP != HGRP - 1:
            flush_heads((NG - 1) // HGRP, (NG - 1) % HGRP + 1)

    nc.compile()
    return nc


_NC = {}


def _get_nc(L):
    if L not in _NC:
        _NC[L] = build_nc(L)
    return _NC[L]


def _segments(done):
    """Split every env's timeline at done=1 into segments, deal segments
    across cores, pack each core's segments into WIDTH column chains.
    Returns (L, plan): L = max chain depth; plan[c] = per-core slot maps."""
    done = np.asarray(done, dtype=np.int32)
    segs = []  # (length, t0, b, initial)
    for b in range(B):
        col = done[:, b]
        starts = np.flatnonzero(col == 1)
        if len(starts) == 0 or starts[0] != 0:
            starts = np.r_[0, starts]
        lens = np.diff(np.r_[starts, T])
        for t0, ln in zip(starts.tolist(), lens.tolist()):
            segs.append((int(ln), int(t0), b, t0 == 0 and col[0] == 0))

    segs.sort(key=lambda s: (-s[0], s[1], s[2]))
    init_segs = [s for s in segs if s[3]]
    rest_segs = [s for s in segs if not s[3]]

    # deal across cores: initial segments round-robin, rest LPT by positions
    core_segs = [[] for _ in range(NCORES)]
    core_load = [0] * NCORES
    for i, s in enumerate(init_segs):
        c = i % NCORES
        core_segs[c].append(s)
        core_load[c] += s[0]
    heap = [(core_load[c], c) for c in range(NCORES)]
    heapq.heapify(heap)
    for s in rest_segs:
        load, c = heapq.heappop(heap)
        core_segs[c].append(s)
        heapq.heappush(heap, (load + s[0], c))

    # pack each core's segments into WIDTH columns (initial segs first, one
    # per column at position 0; then LPT over all columns)
    plan = []
    Lmax = 0
    for c in range(NCORES):
        ini = [s for s in core_segs[c] if s[3]]
        oth = [s for s in core_segs[c] if not s[3]]
        oth.sort(key=lambda s: (-s[0], s[1], s[2]))
        cols = [[] for _ in range(WIDTH)]
        fill = [0] * WIDTH
        for i, s in enumerate(ini):
            cols[i].append(s)
            fill[i] = s[0]
        heap = [(fill[w], w) for w in range(WIDTH)]
        heapq.heapify(heap)
        for s in oth:
            f, w = heapq.heappop(heap)
            cols[w].append(s)
            heapq.heappush(heap, (f + s[0], w))
        Lc = max(sum(s[0] for s in cols[w]) for w in range(WIDTH))
        Lmax = max(Lmax, Lc)
        plan.append({"cols": cols, "n_init": len(ini)})
    L = Lmax

    # build slot maps
    for c in range(NCORES):
        cols = plan[c]["cols"]
        src = np.full((L, WIDTH), -1, dtype=np.int64)
        de = np.ones((L, WIDTH), dtype=np.float32)  # done row (pads -> 1)
        m = np.zeros((L, WIDTH), dtype=np.float32)
        h0b = np.full(WIDTH, -1, dtype=np.int64)  # env idx for init state
        for w in range(WIDTH):
            s_off = 0
            for (ln, t0, b, initial) in cols[w]:
                ts = np.arange(t0, t0 + ln)
                src[s_off:s_off + ln, w] = ts * B + b
                de[s_off, w] = 0.0 if initial else 1.0
                de[s_off + 1:s_off + ln, w] = 0.0
                m[s_off:s_off + ln - 1, w] = 1.0
                if initial:
                    h0b[w] = b
                s_off += ln
        plan[c] = {"src": src, "de": de, "m": m, "h0b": h0b}
    return L, plan


def _make_in_maps(inputs, L, plan):
    import ml_dtypes

    bf16 = ml_dtypes.bfloat16
    x = np.asarray(inputs["x"], dtype=np.float32)
    done = np.asarray(inputs["done"], dtype=np.int32)
    h0 = np.asarray(inputs["h0"], dtype=np.float32).reshape(B, H)
    c0 = np.asarray(inputs["c0"], dtype=np.float32).reshape(B, H)
    Wih = np.asarray(inputs["W_ih"], dtype=np.float32)
    Whh = np.asarray(inputs["W_hh"], dtype=np.float32)
    bias = (np.asarray(inputs["b_ih"], dtype=np.float32)
            + np.asarray(inputs["b_hh"], dtype=np.float32)).reshape(4 * H)
    Wpi = np.asarray(inputs["W_pi"], dtype=np.float32)
    bpi = np.asarray(inputs["b_pi"], dtype=np.float32).reshape(A)
    Wv = np.asarray(inputs["W_v"], dtype=np.float32)
    bv = np.asarray(inputs["b_v"], dtype=np.float32).reshape(1)

    # gate order i,f,g,o -> o,i,f,g; g block (weights + bias) pre-doubled
    order = np.r_[384:512, 0:128, 128:256, 256:384]
    GS = 384  # g block offset after reorder
    FS = 256  # f block offset after reorder
    WihR = Wih[order].copy()
    WihR[GS:GS + 128] *= 2.0
    WhhR = Whh[order].copy()
    WhhR[GS:GS + 128] *= 2.0
    biasR = bias[order].copy()
    biasR[GS:GS + 128] *= 2.0

    wih_aug = np.zeros((INA, 512), dtype=np.float32)
    wih_aug[0:IN] = WihR.T
    wih_aug[IN] = biasR
    wih_aug[IN + 1, FS:FS + 128] = -30.0  # done kills the f gate
    wih_bf = wih_aug.astype(bf16)
    whh_bf = np.ascontiguousarray(WhhR.T).astype(bf16)

    wcat = np.zeros((128, 16), dtype=np.float32)
    wcat[:, 0:A] = Wpi.T
    wcat[:, A] = Wv[0]
    wcat_bf = wcat.astype(bf16)
    bgrp = np.zeros(16, dtype=np.float32)
    bgrp[0:A] = bpi
    bgrp[A] = bv[0]
    bhd = np.tile(bgrp, 32)[None, :].repeat(128, axis=0).copy()  # [128, 512]

    xT = np.ascontiguousarray(x.transpose(2, 0, 1).reshape(IN, T * B))
    h0T = h0.T  # [H, B]
    c0T = c0.T

    in_maps = []
    for c in range(NCORES):
        p = plan[c]
        src = p["src"].reshape(-1)  # [L*WIDTH] in slot order
        valid = src >= 0
        xt = np.zeros((INA, L * WIDTH), dtype=np.float32)
        xt[0:IN, valid] = xT[:, src[valid]]
        xt[IN] = 1.0
        xt[IN + 1] = p["de"].reshape(-1)

        m_bc = np.ascontiguousarray(np.broadcast_to(
            p["m"].reshape(1, L * WIDTH), (128, L * WIDTH))).astype(bf16)

        h0c = np.zeros((H, WIDTH), dtype=np.float32)
        c0c = np.zeros((H, WIDTH), dtype=np.float32)
        wsel = p["h0b"] >= 0
        h0c[:, wsel] = h0T[:, p["h0b"][wsel]]
        c0c[:, wsel] = c0T[:, p["h0b"][wsel]]

        in_maps.append({
            "xt": xt.astype(bf16),
            "m": m_bc,
            "h0": h0c.astype(bf16),
            "c0": c0c.astype(bf16),
            "wih": wih_bf,
            "whh": whh_bf,
            "wcat": wcat_bf,
            "bhd": bhd,
        })
    return in_maps


def _try_device_reset():
    try:
        import ctypes

        import jax

        jax.devices()
        lib = ctypes.CDLL("/opt/axon/libaxon_pjrt.so")
        if hasattr(lib, "axon_reset"):
            lib.axon_reset.restype = ctypes.c_int64
            lib.axon_reset()
    except Exception:
        pass


def kernel(**inputs):
    from concourse.bass_utils import run_bass_kernel_spmd

    done = np.asarray(inputs["done"], dtype=np.int32)
    L, plan = _segments(done)
    nc = _get_nc(L)
    in_maps = _make_in_maps(inputs, L, plan)
    try:
        res = run_bass_kernel_spmd(nc, in_maps, core_ids=list(range(NCORES)))
    except Exception:
        _try_device_reset()
        res = run_bass_kernel_spmd(nc, in_maps, core_ids=list(range(NCORES)))
    full = np.empty((T * B, NOUT), dtype=np.float32)
    for c in range(NCORES):
        out = res.results[c]["out"]  # [NG*CH, 16] in slot order
        src = plan[c]["src"].reshape(-1)
        valid = src >= 0
        full[src[valid]] = out[valid][:, 0:NOUT]
    return full


# revision 19
# speedup vs baseline: 2.2882x; 1.0518x over previous
"""Trainium2 Bass kernel for nn_ActorCritic (LSTM with done-resets + heads).

Segment-packed formulation. done ~ Bernoulli(0.5) per (t, env) resets (h, c)
at the START of step t, so the T=512 scan factorizes into ~T*B/2 independent
segments (mean length 2, max ~18). Host-side we split every env's timeline
into segments, deal them across the 8 cores (LPT by positions), and bin-pack
each core's segments into WIDTH=768 column chains of depth L (~22). This is
EXACT for any input (no warmup redundancy): resets inside a chain are handled
by the baseline's -30*done f-gate kill (c history) and the m mask (h history),
and chain position 0 gets (h0, c0) for columns seeded with an env's initial
segment. Serial depth drops 72 -> ~22 while every macro-step stays 768 wide.

Device per core, per macro-step: 3 independent 256-col chunks (global chunk
index g = 3*s + j; the recurrence chain is g -> g+3, so each engine always
has ~2 chunks of unrelated work to hide the chain latency):
  - xW GEMM (3 K-tiles x 4 gates, 256-col pieces) streams 2 chunks ahead
    into the chunk's [128,1024] PSUM tile (2 banks, one start per bank);
    4 W_hh matmuls (N=256) accumulate on top.
  - ONE sigmoid over the whole [128,1024] gate tile (slot order [o,i,f,g],
    g block pre-doubled so tanh(g) = 2*sig(2g)-1), ONE tanh per chunk.
  - DVE tail (bf16 2x): t2 = sig_f*c; u = (sig_g'-0.5)*sig_i; c = 2u+t2;
    hs = sig_o*tanh(c) into the bf16 history; hm = hs*m for the recurrence.
  - Heads: 2 matmuls per chunk accumulate into a [128,512] PSUM bank shared
    by 16 chunks; one fused bias-add + one DMA per 16 chunks.

Host-side marshalling (not compute): segment packing, x gather into packed
column order (bf16, +ones row for bias, +done row for the f-kill), m mask
broadcast, output scatter back to (t, env) order.
"""

import heapq
import sys
from contextlib import ExitStack

import numpy as np

sys.path.insert(0, "/opt/trn_rl_repo")

# Problem constants (hardcoded per harness contract).
T = 512
B = 256
NCORES = 8
IN = 292
H = 128
A = 12
NOUT = 13

INA = IN + 2  # +ones row (bias), +done row (f-gate kill)
KSPLITS = [(0, 128), (128, 128)]  # full-height xW k-pieces
K3 = 256  # third piece rows [256:294] run row-paired via tile_position
K3R = INA - K3  # 38
CH = 256  # chunk width (cols)
NJ = 3  # chunks per macro-step
WIDTH = NJ * CH  # column chains per core
GCH = 24  # max g-chunks per input slab (SBUF budget)


def _chunks(NG):
    # small first slabs so the first xW can start ~10us earlier
    sizes = []
    left = NG
    for first in (4, 12):
        n = min(first, left)
        if n:
            sizes.append(n)
        left -= n
    while left > 0:
        n = min(GCH, left)
        sizes.append(n)
        left -= n
    return sizes


def build_nc(L):
    import concourse.bass as bass
    import concourse.tile as tile
    from concourse import bacc, mybir

    f32 = mybir.dt.float32
    bf16 = mybir.dt.bfloat16
    AF = mybir.ActivationFunctionType
    OP = mybir.AluOpType

    NG = NJ * L
    gchs = _chunks(NG)
    NCH = len(gchs)
    coff = [0]
    for n in gchs:
        coff.append(coff[-1] + n)
    g_slab = []
    for ch, n in enumerate(gchs):
        g_slab += [ch] * n

    nc = bacc.Bacc("TRN2", target_bir_lowering=False, debug=False)

    # ---- I/O (all per-core slices prepared by host) ----
    xt_d = nc.dram_tensor("xt", [INA, NG * CH], bf16, kind="ExternalInput").ap()
    m_d = nc.dram_tensor("m", [128, NG * CH], bf16, kind="ExternalInput").ap()
    h0_d = nc.dram_tensor("h0", [128, WIDTH], bf16, kind="ExternalInput").ap()
    c0_d = nc.dram_tensor("c0", [128, WIDTH], bf16, kind="ExternalInput").ap()
    wih_d = nc.dram_tensor("wih", [INA, 512], bf16, kind="ExternalInput").ap()
    wih3_d = nc.dram_tensor("wih3", [102, 512], bf16, kind="ExternalInput").ap()
    whh_d = nc.dram_tensor("whh", [128, 512], bf16, kind="ExternalInput").ap()
    wcat_d = nc.dram_tensor("wcat", [128, 16], bf16, kind="ExternalInput").ap()
    bhd_d = nc.dram_tensor("bhd", [16, 512], f32, kind="ExternalInput").ap()
    out_d = nc.dram_tensor("out", [16, NG * CH], f32, kind="ExternalOutput").ap()

    with tile.TileContext(nc) as tc, ExitStack() as ctx:
        cst = ctx.enter_context(tc.tile_pool(name="cst", bufs=1))
        big = ctx.enter_context(tc.tile_pool(name="big", bufs=1))
        xtp = ctx.enter_context(tc.tile_pool(name="xtp", bufs=2))
        mp = ctx.enter_context(tc.tile_pool(name="mp", bufs=2))
        wk = ctx.enter_context(tc.tile_pool(name="wk", bufs=3))
        pg_pool = ctx.enter_context(tc.tile_pool(name="pg", bufs=3, space="PSUM"))
        php = ctx.enter_context(tc.tile_pool(name="ph", bufs=2, space="PSUM"))

        # ---- persistent tiles ----
        wih_sb = [cst.tile([sz, 512], bf16, tag=f"wih{k}", name=f"wih{k}")
                  for k, (_, sz) in enumerate(KSPLITS)]
        # k3 piece duplicated at partitions 0:38 and 64:102 so the 4 gate
        # slots run as 2 concurrent row-tile pairs (tile_position via
        # base_partition) into different PSUM banks.
        wih3_sb = cst.tile([102, 512], bf16, tag="wih3", name="wih3")
        whh_sb = cst.tile([128, 512], bf16, tag="whh", name="whh")
        wcat_sb = cst.tile([128, 16], bf16, tag="wcat", name="wcat")
        bhd_sb = cst.tile([16, 512], f32, tag="bhd", name="bhd")
        h0_sb = cst.tile([128, WIDTH], bf16, tag="h0", name="h0")
        c0_sb = cst.tile([128, WIDTH], bf16, tag="c0", name="c0")
        hs_all = big.tile([128, NG * CH], bf16, tag="hs", name="hs")

        for k, (off, sz) in enumerate(KSPLITS):
            nc.sync.dma_start(out=wih_sb[k][:, :], in_=wih_d[off:off + sz, :])
        nc.sync.dma_start(out=wih3_sb[:, :], in_=wih3_d[:, :])
        nc.sync.dma_start(out=whh_sb[:, :], in_=whh_d[:, :])
        nc.sync.dma_start(out=wcat_sb[:, :], in_=wcat_d[:, :])
        nc.sync.dma_start(out=bhd_sb[:, :], in_=bhd_d[:, :])
        nc.sync.dma_start(out=h0_sb[:, :], in_=h0_d[:, :])
        nc.sync.dma_start(out=c0_sb[:, :], in_=c0_d[:, :])

        # ---- input slab DMAs ----
        xts = {}
        mts = {}

        def load_slab(ch):
            if ch >= NCH:
                return
            n = gchs[ch] * CH
            c0_, c1_ = coff[ch] * CH, coff[ch + 1] * CH
            h = n // 2  # split DMAs for queue parallelism
            tiles = []
            for k, (off, sz) in enumerate(KSPLITS):
                xt = xtp.tile([sz, GCH * CH], bf16, tag=f"xt{k}", name=f"xt{k}")
                nc.sync.dma_start(out=xt[:, 0:h],
                                  in_=xt_d[off:off + sz, c0_:c0_ + h])
                nc.sync.dma_start(out=xt[:, h:n],
                                  in_=xt_d[off:off + sz, c0_ + h:c1_])
                tiles.append(xt)
            # k3 rows duplicated at partition bases 0 and 64 (row pairing)
            xt3 = xtp.tile([102, GCH * CH], bf16, tag="xt3", name="xt3")
            for pb in (0, 64):
                nc.sync.dma_start(out=xt3[pb:pb + K3R, 0:h],
                                  in_=xt_d[K3:INA, c0_:c0_ + h])
                nc.sync.dma_start(out=xt3[pb:pb + K3R, h:n],
                                  in_=xt_d[K3:INA, c0_ + h:c1_])
            tiles.append(xt3)
            xts[ch] = tiles
            mt = mp.tile([128, GCH * CH], bf16, tag="mt", name="mt")
            nc.sync.dma_start(out=mt[:, 0:h], in_=m_d[:, c0_:c0_ + h])
            nc.sync.dma_start(out=mt[:, h:n], in_=m_d[:, c0_ + h:c1_])
            mts[ch] = mt

        load_slab(0)
        load_slab(1)

        # ---- PE warmup + ACT table preload ----
        # HAM starts cold (1.2 GHz) and needs ~3.4us of sustained PE work to
        # un-throttle; dummy matmuls on the (small, early) whh tile warm it
        # while the first x slab DMA is in flight. The sigmoid table load
        # (~2.7us) is also hoisted off the first chunk's critical path.
        scr = php.tile([16, 512], f32, tag="ph", name="scr")
        for i in range(14):
            nc.tensor.matmul(scr[:, :], whh_sb[:, 0:16], whh_sb[0:128, :],
                             start=True, stop=False)
        dumm = wk.tile([128, 16], bf16, tag="dumm", name="dumm")
        nc.scalar.activation(dumm[:, :], h0_sb[:, 0:16], AF.Sigmoid)

        def fillers(k):
            for _ in range(k):
                nc.tensor.matmul(scr[:, :], whh_sb[:, 0:16],
                                 whh_sb[0:128, :], start=True, stop=False)

        # ---- xW GEMM straight into the chunk's PSUM tile ----
        # One [128, 4*256] tile per chunk (2 banks; cols = slot*256 + c).
        # Slot order [o, i, f, g]. ONE start per 2KB bank (slot 0 / slot 2
        # first k-piece); later slots' first writes auto-zero, accumulation
        # is per-element via has_written.
        psum_tiles = {}
        PIECES = [(slot, k) for slot in range(4) for k in range(len(KSPLITS))]

        def emit_xw(g):
            if g >= NG:
                return
            pg = pg_pool.tile([128, 1024], f32, tag="pg", name="pg")
            psum_tiles[g] = pg
            tiles = xts[g_slab[g]]
            c0_ = (g - coff[g_slab[g]]) * CH
            for slot, k in PIECES:
                off, sz = KSPLITS[k]
                nc.tensor.matmul(
                    pg[:, slot * CH:slot * CH + CH],
                    wih_sb[k][0:sz, slot * 128:(slot + 1) * 128],
                    tiles[k][0:sz, c0_:c0_ + CH],
                    start=(slot in (0, 2) and k == 0), stop=False)
            # k3 pieces as concurrent row-tile pairs: (slot0@row0, slot2@row64)
            # and (slot1@row0, slot3@row64) target different PSUM banks, so
            # each pair streams in ~256 shared cycles instead of 2x256.
            xt3 = tiles[2]
            for s01 in (0, 1):
                for pb, slot in ((0, s01), (64, s01 + 2)):
                    nc.tensor.matmul(
                        pg[:, slot * CH:slot * CH + CH],
                        wih3_sb[pb:pb + K3R, slot * 128:(slot + 1) * 128],
                        xt3[pb:pb + K3R, c0_:c0_ + CH],
                        start=False, stop=False)

        # heads: wcat stationary (16-col LDW), hs moving; out [16, NG*CH]
        # transposed (host untransposes). 2 chunks share one PSUM bank.
        ph_tiles = {}

        def emit_heads(g):
            pair, loc = g // 2, g % 2
            if loc == 0:
                ph_tiles[pair] = php.tile([16, 512], f32, tag="ph", name="ph")
            ph = ph_tiles[pair]
            nc.tensor.matmul(ph[:, loc * CH:loc * CH + CH],
                             wcat_sb[:, :], hs_all[:, g * CH:g * CH + CH],
                             start=(loc == 0), stop=(loc == 1 or g == NG - 1))

        def flush_heads(pair):
            # fused bias-add + PSUM->SBUF copy, one DMA per chunk pair
            ph = ph_tiles.pop(pair)
            n = min(512, NG * CH - pair * 512)
            ob = wk.tile([16, 512], f32, tag="ob", name="ob")
            nc.vector.scalar_tensor_tensor(
                ob[:, 0:n], ph[:, 0:n], 1.0, bhd_sb[:, 0:n], OP.mult, OP.add)
            nc.sync.dma_start(out=out_d[:, pair * 512:pair * 512 + n],
                              in_=ob[:, 0:n])

        emit_xw(0)
        emit_xw(1)

        hm_ref = {}
        c_ref = {}
        for j in range(NJ):
            hm_ref[j - NJ] = h0_sb[:, j * CH:(j + 1) * CH]
            c_ref[j - NJ] = c0_sb[:, j * CH:(j + 1) * CH]

        sig_ref = {}
        th_ref = {}

        # ---- the recurrence over global chunks ----
        for g in range(NG):
            if g > 0 and g - 1 in coff:
                load_slab(coff.index(g - 1) + 2)
            mt = mts[g_slab[g]]
            mc = (g - coff[g_slab[g]]) * CH

            # PE: W_hh accumulation for chunk g (chain input from g-3)
            pg = psum_tiles.pop(g)
            hm_in = hm_ref.pop(g - NJ)
            for slot in range(4):
                nc.tensor.matmul(
                    pg[:, slot * CH:slot * CH + CH],
                    whh_sb[:, slot * 128:(slot + 1) * 128],
                    hm_in, start=False, stop=(slot in (1, 3)))

            # PE fillers: xW two chunks ahead, heads two chunks behind
            emit_xw(g + 2)
            if g < 9:
                # keep PE busy/warm through pipeline priming
                fillers(6 if g == 0 else 4 if g < 4 else 2)
            if g >= 2:
                emit_heads(g - 2)

            # ACT: tanh of last chunk's c (input was ready long ago), then
            # the one big sigmoid for this chunk.
            if g >= 1:
                th = wk.tile([128, CH], bf16, tag="th", name="th")
                nc.scalar.activation(th[:, :], c_ref[g - 1][:, :], AF.Tanh)
                th_ref[g - 1] = th
            sig = wk.tile([128, 1024], bf16, tag="sig", name="sig")
            nc.scalar.activation(sig[:, :], pg[:, :], AF.Sigmoid)
            sig_ref[g] = sig

            # DVE: finish chunk g-1 (hs into history, masked hm for the
            # chain), then chunk g's c-path.
            if g >= 1:
                sp = sig_ref[g - 1]
                col = (g - 1) * CH
                nc.vector.tensor_mul(hs_all[:, col:col + CH],
                                     sp[:, 0:CH], th_ref.pop(g - 1)[:, :])
                if (g - 1) // NJ < L - 1:
                    hm = wk.tile([128, CH], bf16, tag="hm", name="hm")
                    nc.vector.tensor_mul(hm[:, :], hs_all[:, col:col + CH],
                                         mts[g_slab[g - 1]][
                                             :, (g - 1 - coff[g_slab[g - 1]])
                                             * CH:(g - coff[g_slab[g - 1]]) * CH])
                    hm_ref[g - 1] = hm
                del sig_ref[g - 1]
            t2 = wk.tile([128, CH], bf16, tag="t2", name="t2")
            nc.vector.tensor_mul(t2[:, :], sig[:, 2 * CH:3 * CH],
                                 c_ref.pop(g - NJ)[:, :])
            u = wk.tile([128, CH], bf16, tag="u", name="u")
            nc.vector.scalar_tensor_tensor(
                u[:, :], sig[:, 3 * CH:4 * CH], 0.5, sig[:, CH:2 * CH],
                OP.subtract, OP.mult)
            cn = wk.tile([128, CH], bf16, tag="cn", name="cn")
            nc.vector.scalar_tensor_tensor(
                cn[:, :], u[:, :], 2.0, t2[:, :], OP.mult, OP.add)
            c_ref[g] = cn
            if g >= 2 and (g - 2) % 2 == 1:
                flush_heads((g - 2) // 2)

        # ---- drain: last chunk's tanh/hs, remaining heads ----
        th = wk.tile([128, CH], bf16, tag="th", name="th")
        nc.scalar.activation(th[:, :], c_ref[NG - 1][:, :], AF.Tanh)
        nc.vector.tensor_mul(hs_all[:, (NG - 1) * CH:NG * CH],
                             sig_ref[NG - 1][:, 0:CH], th[:, :])
        for g in (NG - 2, NG - 1):
            emit_heads(g)
            if g % 2 == 1 or g == NG - 1:
                flush_heads(g // 2)

    nc.compile()
    return nc


_NC = {}


def _get_nc(L):
    if L not in _NC:
        _NC[L] = build_nc(L)
    return _NC[L]


def _segments(done):
    """Split every env's timeline at done=1 into segments, deal segments
    across cores, pack each core's segments into WIDTH column chains.
    Returns (L, plan): L = max chain depth; plan[c] = per-core slot maps."""
    done = np.asarray(done, dtype=np.int32)
    segs = []  # (length, t0, b, initial)
    for b in range(B):
        col = done[:, b]
        starts = np.flatnonzero(col == 1)
        if len(starts) == 0 or starts[0] != 0:
            starts = np.r_[0, starts]
        lens = np.diff(np.r_[starts, T])
        for t0, ln in zip(starts.tolist(), lens.tolist()):
            segs.append((int(ln), int(t0), b, t0 == 0 and col[0] == 0))

    segs.sort(key=lambda s: (-s[0], s[1], s[2]))
    init_segs = [s for s in segs if s[3]]
    rest_segs = [s for s in segs if not s[3]]

    # deal across cores: initial segments round-robin, rest LPT by positions
    core_segs = [[] for _ in range(NCORES)]
    core_load = [0] * NCORES
    for i, s in enumerate(init_segs):
        c = i % NCORES
        core_segs[c].append(s)
        core_load[c] += s[0]
    heap = [(core_load[c], c) for c in range(NCORES)]
    heapq.heapify(heap)
    for s in rest_segs:
        load, c = heapq.heappop(heap)
        core_segs[c].append(s)
        heapq.heappush(heap, (load + s[0], c))

    # pack each core's segments into WIDTH columns (initial segs first, one
    # per column at position 0; then LPT over all columns)
    plan = []
    Lmax = 0
    for c in range(NCORES):
        ini = [s for s in core_segs[c] if s[3]]
        oth = [s for s in core_segs[c] if not s[3]]
        oth.sort(key=lambda s: (-s[0], s[1], s[2]))
        cols = [[] for _ in range(WIDTH)]
        fill = [0] * WIDTH
        for i, s in enumerate(ini):
            cols[i].append(s)
            fill[i] = s[0]
        heap = [(fill[w], w) for w in range(WIDTH)]
        heapq.heapify(heap)
        for s in oth:
            f, w = heapq.heappop(heap)
            cols[w].append(s)
            heapq.heappush(heap, (f + s[0], w))
        Lc = max(sum(s[0] for s in cols[w]) for w in range(WIDTH))
        Lmax = max(Lmax, Lc)
        plan.append({"cols": cols, "n_init": len(ini)})
    L = Lmax

    # build slot maps
    for c in range(NCORES):
        cols = plan[c]["cols"]
        src = np.full((L, WIDTH), -1, dtype=np.int64)
        de = np.ones((L, WIDTH), dtype=np.float32)  # done row (pads -> 1)
        m = np.zeros((L, WIDTH), dtype=np.float32)
        h0b = np.full(WIDTH, -1, dtype=np.int64)  # env idx for init state
        for w in range(WIDTH):
            s_off = 0
            for (ln, t0, b, initial) in cols[w]:
                ts = np.arange(t0, t0 + ln)
                src[s_off:s_off + ln, w] = ts * B + b
                de[s_off, w] = 0.0 if initial else 1.0
                de[s_off + 1:s_off + ln, w] = 0.0
                m[s_off:s_off + ln - 1, w] = 1.0
                if initial:
                    h0b[w] = b
                s_off += ln
        plan[c] = {"src": src, "de": de, "m": m, "h0b": h0b}
    return L, plan


def _make_in_maps(inputs, L, plan):
    import ml_dtypes

    bf16 = ml_dtypes.bfloat16
    x = np.asarray(inputs["x"], dtype=np.float32)
    done = np.asarray(inputs["done"], dtype=np.int32)
    h0 = np.asarray(inputs["h0"], dtype=np.float32).reshape(B, H)
    c0 = np.asarray(inputs["c0"], dtype=np.float32).reshape(B, H)
    Wih = np.asarray(inputs["W_ih"], dtype=np.float32)
    Whh = np.asarray(inputs["W_hh"], dtype=np.float32)
    bias = (np.asarray(inputs["b_ih"], dtype=np.float32)
            + np.asarray(inputs["b_hh"], dtype=np.float32)).reshape(4 * H)
    Wpi = np.asarray(inputs["W_pi"], dtype=np.float32)
    bpi = np.asarray(inputs["b_pi"], dtype=np.float32).reshape(A)
    Wv = np.asarray(inputs["W_v"], dtype=np.float32)
    bv = np.asarray(inputs["b_v"], dtype=np.float32).reshape(1)

    # gate order i,f,g,o -> o,i,f,g; g block (weights + bias) pre-doubled
    order = np.r_[384:512, 0:128, 128:256, 256:384]
    GS = 384  # g block offset after reorder
    FS = 256  # f block offset after reorder
    WihR = Wih[order].copy()
    WihR[GS:GS + 128] *= 2.0
    WhhR = Whh[order].copy()
    WhhR[GS:GS + 128] *= 2.0
    biasR = bias[order].copy()
    biasR[GS:GS + 128] *= 2.0

    wih_aug = np.zeros((INA, 512), dtype=np.float32)
    wih_aug[0:IN] = WihR.T
    wih_aug[IN] = biasR
    wih_aug[IN + 1, FS:FS + 128] = -30.0  # done kills the f gate
    wih_bf = wih_aug.astype(bf16)
    wih3 = np.zeros((102, 512), dtype=np.float32)
    wih3[0:K3R] = wih_aug[K3:INA]  # duplicated at partition bases 0 and 64
    wih3[64:64 + K3R] = wih_aug[K3:INA]
    wih3_bf = wih3.astype(bf16)
    whh_bf = np.ascontiguousarray(WhhR.T).astype(bf16)

    wcat = np.zeros((128, 16), dtype=np.float32)
    wcat[:, 0:A] = Wpi.T
    wcat[:, A] = Wv[0]
    wcat_bf = wcat.astype(bf16)
    bgrp = np.zeros(16, dtype=np.float32)
    bgrp[0:A] = bpi
    bgrp[A] = bv[0]
    bhd = np.ascontiguousarray(
        np.broadcast_to(bgrp[:, None], (16, 512)))  # [16, 512]

    xT = np.ascontiguousarray(x.transpose(2, 0, 1).reshape(IN, T * B))
    h0T = h0.T  # [H, B]
    c0T = c0.T

    in_maps = []
    for c in range(NCORES):
        p = plan[c]
        src = p["src"].reshape(-1)  # [L*WIDTH] in slot order
        valid = src >= 0
        xt = np.zeros((INA, L * WIDTH), dtype=np.float32)
        xt[0:IN, valid] = xT[:, src[valid]]
        xt[IN] = 1.0
        xt[IN + 1] = p["de"].reshape(-1)

        m_bc = np.ascontiguousarray(np.broadcast_to(
            p["m"].reshape(1, L * WIDTH), (128, L * WIDTH))).astype(bf16)

        h0c = np.zeros((H, WIDTH), dtype=np.float32)
        c0c = np.zeros((H, WIDTH), dtype=np.float32)
        wsel = p["h0b"] >= 0
        h0c[:, wsel] = h0T[:, p["h0b"][wsel]]
        c0c[:, wsel] = c0T[:, p["h0b"][wsel]]

        in_maps.append({
            "xt": xt.astype(bf16),
            "m": m_bc,
            "h0": h0c.astype(bf16),
            "c0": c0c.astype(bf16),
            "wih": wih_bf,
            "wih3": wih3_bf,
            "whh": whh_bf,
            "wcat": wcat_bf,
            "bhd": bhd,
        })
    return in_maps


def _try_device_reset():
    try:
        import ctypes

        import jax

        jax.devices()
        lib = ctypes.CDLL("/opt/axon/libaxon_pjrt.so")
        if hasattr(lib, "axon_reset"):
            lib.axon_reset.restype = ctypes.c_int64
            lib.axon_reset()
    except Exception:
        pass


def kernel(**inputs):
    from concourse.bass_utils import run_bass_kernel_spmd

    done = np.asarray(inputs["done"], dtype=np.int32)
    L, plan = _segments(done)
    nc = _get_nc(L)
    in_maps = _make_in_maps(inputs, L, plan)
    try:
        res = run_bass_kernel_spmd(nc, in_maps, core_ids=list(range(NCORES)))
    except Exception:
        _try_device_reset()
        res = run_bass_kernel_spmd(nc, in_maps, core_ids=list(range(NCORES)))
    full = np.empty((T * B, NOUT), dtype=np.float32)
    for c in range(NCORES):
        out = np.ascontiguousarray(res.results[c]["out"].T)  # [NG*CH, 16]
        src = plan[c]["src"].reshape(-1)
        valid = src >= 0
        full[src[valid]] = out[valid][:, 0:NOUT]
    return full


# revision 21
# speedup vs baseline: 2.5950x; 1.1341x over previous
"""Trainium2 Bass kernel for nn_ActorCritic (LSTM with done-resets + heads).

Segment-packed formulation. done ~ Bernoulli(0.5) per (t, env) resets (h, c)
at the START of step t, so the T=512 scan factorizes into ~T*B/2 independent
segments (mean length 2, max ~18). Host-side we split every env's timeline
into segments, deal them across the 8 cores (LPT by positions), and bin-pack
each core's segments into WIDTH=768 column chains of depth L (~22). This is
EXACT for any input (no warmup redundancy): resets inside a chain are handled
by the baseline's -30*done f-gate kill (c history) and the m mask (h history),
and chain position 0 gets (h0, c0) for columns seeded with an env's initial
segment. Serial depth drops 72 -> ~22 while every macro-step stays 768 wide.

Device per core, per macro-step: 3 independent 256-col chunks (global chunk
index g = 3*s + j; the recurrence chain is g -> g+3, so each engine always
has ~2 chunks of unrelated work to hide the chain latency):
  - xW GEMM (3 K-tiles x 4 gates, 256-col pieces) streams 2 chunks ahead
    into the chunk's [128,1024] PSUM tile (2 banks, one start per bank);
    4 W_hh matmuls (N=256) accumulate on top.
  - ONE sigmoid over the whole [128,1024] gate tile (slot order [o,i,f,g],
    g block pre-doubled so tanh(g) = 2*sig(2g)-1), ONE tanh per chunk.
  - DVE tail (bf16 2x): t2 = sig_f*c; u = (sig_g'-0.5)*sig_i; c = 2u+t2;
    hs = sig_o*tanh(c) into the bf16 history; hm = hs*m for the recurrence.
  - Heads: 2 matmuls per chunk accumulate into a [128,512] PSUM bank shared
    by 16 chunks; one fused bias-add + one DMA per 16 chunks.

Host-side marshalling (not compute): segment packing, x gather into packed
column order (bf16, +ones row for bias, +done row for the f-kill), m mask
broadcast, output scatter back to (t, env) order.
"""

import heapq
import sys
from contextlib import ExitStack

import numpy as np

sys.path.insert(0, "/opt/trn_rl_repo")

# Problem constants (hardcoded per harness contract).
T = 512
B = 256
NCORES = 8
IN = 292
H = 128
A = 12
NOUT = 13

INA = IN + 2  # +ones row (bias), +done row (f-gate kill)
KSPLITS = [(0, 128), (128, 128)]  # full-height xW k-pieces
K3 = 256  # third piece rows [256:294] run row-paired via tile_position
K3R = INA - K3  # 38
CH = 256  # chunk width (cols)
NJ = 3  # chunks per macro-step
WIDTH = NJ * CH  # column chains per core
GCH = 24  # max g-chunks per input slab (SBUF budget)


def _chunks(NG):
    # small first slabs so the first xW can start ~10us earlier
    sizes = []
    left = NG
    for first in (4, 12):
        n = min(first, left)
        if n:
            sizes.append(n)
        left -= n
    while left > 0:
        n = min(GCH, left)
        sizes.append(n)
        left -= n
    return sizes


def build_nc(L):
    import concourse.bass as bass
    import concourse.tile as tile
    from concourse import bacc, mybir

    f32 = mybir.dt.float32
    bf16 = mybir.dt.bfloat16
    AF = mybir.ActivationFunctionType
    OP = mybir.AluOpType

    NG = NJ * L
    gchs = _chunks(NG)
    NCH = len(gchs)
    coff = [0]
    for n in gchs:
        coff.append(coff[-1] + n)
    g_slab = []
    for ch, n in enumerate(gchs):
        g_slab += [ch] * n

    nc = bacc.Bacc("TRN2", target_bir_lowering=False, debug=False)

    # ---- I/O (all per-core slices prepared by host) ----
    xt_d = nc.dram_tensor("xt", [INA, NG * CH], bf16, kind="ExternalInput").ap()
    m_d = nc.dram_tensor("m", [128, NG * CH], bf16, kind="ExternalInput").ap()
    h0_d = nc.dram_tensor("h0", [128, WIDTH], bf16, kind="ExternalInput").ap()
    c0_d = nc.dram_tensor("c0", [128, WIDTH], bf16, kind="ExternalInput").ap()
    wih_d = nc.dram_tensor("wih", [INA, 512], bf16, kind="ExternalInput").ap()
    wih3_d = nc.dram_tensor("wih3", [102, 512], bf16, kind="ExternalInput").ap()
    whh_d = nc.dram_tensor("whh", [128, 512], bf16, kind="ExternalInput").ap()
    wcat_d = nc.dram_tensor("wcat", [128, 16], bf16, kind="ExternalInput").ap()
    bhd_d = nc.dram_tensor("bhd", [16, 512], f32, kind="ExternalInput").ap()
    out_d = nc.dram_tensor("out", [16, NG * CH], f32, kind="ExternalOutput").ap()

    with tile.TileContext(nc) as tc, ExitStack() as ctx:
        cst = ctx.enter_context(tc.tile_pool(name="cst", bufs=1))
        big = ctx.enter_context(tc.tile_pool(name="big", bufs=1))
        xtp = ctx.enter_context(tc.tile_pool(name="xtp", bufs=2))
        mp = ctx.enter_context(tc.tile_pool(name="mp", bufs=2))
        wk = ctx.enter_context(tc.tile_pool(name="wk", bufs=3))
        pg_pool = ctx.enter_context(tc.tile_pool(name="pg", bufs=3, space="PSUM"))
        php = ctx.enter_context(tc.tile_pool(name="ph", bufs=2, space="PSUM"))

        # ---- persistent tiles ----
        wih_sb = [cst.tile([sz, 512], bf16, tag=f"wih{k}", name=f"wih{k}")
                  for k, (_, sz) in enumerate(KSPLITS)]
        # k3 piece duplicated at partitions 0:38 and 64:102 so the 4 gate
        # slots run as 2 concurrent row-tile pairs (tile_position via
        # base_partition) into different PSUM banks.
        wih3_sb = cst.tile([102, 512], bf16, tag="wih3", name="wih3")
        whh_sb = cst.tile([128, 512], bf16, tag="whh", name="whh")
        wcat_sb = cst.tile([128, 16], bf16, tag="wcat", name="wcat")
        bhd_sb = cst.tile([16, 512], f32, tag="bhd", name="bhd")
        h0_sb = cst.tile([128, WIDTH], bf16, tag="h0", name="h0")
        c0_sb = cst.tile([128, WIDTH], bf16, tag="c0", name="c0")
        hs_all = big.tile([128, NG * CH], bf16, tag="hs", name="hs")

        for k, (off, sz) in enumerate(KSPLITS):
            nc.sync.dma_start(out=wih_sb[k][:, :], in_=wih_d[off:off + sz, :])
        nc.sync.dma_start(out=wih3_sb[:, :], in_=wih3_d[:, :])
        nc.sync.dma_start(out=whh_sb[:, :], in_=whh_d[:, :])
        nc.sync.dma_start(out=wcat_sb[:, :], in_=wcat_d[:, :])
        nc.sync.dma_start(out=bhd_sb[:, :], in_=bhd_d[:, :])
        nc.sync.dma_start(out=h0_sb[:, :], in_=h0_d[:, :])
        nc.sync.dma_start(out=c0_sb[:, :], in_=c0_d[:, :])

        # ---- input slab DMAs ----
        xts = {}
        mts = {}

        def load_slab(ch):
            if ch >= NCH:
                return
            n = gchs[ch] * CH
            c0_, c1_ = coff[ch] * CH, coff[ch + 1] * CH
            h = n // 2
            # spread across the 3 DMA-capable queues: gpsimd (SWDGE),
            # sync (HWDGE), scalar (HWDGE) -- a single queue tops out at
            # ~90GB/s and stalls the PE at slab boundaries.
            tiles = []
            for k, (off, sz) in enumerate(KSPLITS):
                eng = nc.gpsimd if k == 0 else nc.sync
                xt = xtp.tile([sz, GCH * CH], bf16, tag=f"xt{k}", name=f"xt{k}")
                eng.dma_start(out=xt[:, 0:h],
                              in_=xt_d[off:off + sz, c0_:c0_ + h])
                eng.dma_start(out=xt[:, h:n],
                              in_=xt_d[off:off + sz, c0_ + h:c1_])
                tiles.append(xt)
            # k3 rows duplicated at partition bases 0 and 64 (row pairing)
            xt3 = xtp.tile([102, GCH * CH], bf16, tag="xt3", name="xt3")
            for pb in (0, 64):
                nc.gpsimd.dma_start(out=xt3[pb:pb + K3R, 0:h],
                                    in_=xt_d[K3:INA, c0_:c0_ + h])
                nc.gpsimd.dma_start(out=xt3[pb:pb + K3R, h:n],
                                    in_=xt_d[K3:INA, c0_ + h:c1_])
            tiles.append(xt3)
            xts[ch] = tiles
            mt = mp.tile([128, GCH * CH], bf16, tag="mt", name="mt")
            nc.sync.dma_start(out=mt[:, 0:h], in_=m_d[:, c0_:c0_ + h])
            nc.scalar.dma_start(out=mt[:, h:n], in_=m_d[:, c0_ + h:c1_])
            mts[ch] = mt

        load_slab(0)
        load_slab(1)

        # ---- PE warmup + ACT table preload ----
        # HAM starts cold (1.2 GHz) and needs ~3.4us of sustained PE work to
        # un-throttle; dummy matmuls on the (small, early) whh tile warm it
        # while the first x slab DMA is in flight. The sigmoid table load
        # (~2.7us) is also hoisted off the first chunk's critical path.
        scr = php.tile([16, 512], f32, tag="ph", name="scr")
        for i in range(14):
            nc.tensor.matmul(scr[:, :], whh_sb[:, 0:16], whh_sb[0:128, :],
                             start=True, stop=False)
        dumm = wk.tile([128, 16], bf16, tag="dumm", name="dumm")
        nc.scalar.activation(dumm[:, :], h0_sb[:, 0:16], AF.Sigmoid)

        def fillers(k):
            for _ in range(k):
                nc.tensor.matmul(scr[:, :], whh_sb[:, 0:16],
                                 whh_sb[0:128, :], start=True, stop=False)

        # ---- xW GEMM straight into the chunk's PSUM tile ----
        # One [128, 4*256] tile per chunk (2 banks; cols = slot*256 + c).
        # Slot order [o, i, f, g]. ONE start per 2KB bank (slot 0 / slot 2
        # first k-piece); later slots' first writes auto-zero, accumulation
        # is per-element via has_written.
        psum_tiles = {}
        PIECES = [(slot, k) for slot in range(4) for k in range(len(KSPLITS))]

        def emit_xw(g):
            if g >= NG:
                return
            pg = pg_pool.tile([128, 1024], f32, tag="pg", name="pg")
            psum_tiles[g] = pg
            tiles = xts[g_slab[g]]
            c0_ = (g - coff[g_slab[g]]) * CH
            for slot, k in PIECES:
                off, sz = KSPLITS[k]
                nc.tensor.matmul(
                    pg[:, slot * CH:slot * CH + CH],
                    wih_sb[k][0:sz, slot * 128:(slot + 1) * 128],
                    tiles[k][0:sz, c0_:c0_ + CH],
                    start=(slot in (0, 2) and k == 0), stop=False)
            # k3 pieces as concurrent row-tile pairs: (slot0@row0, slot2@row64)
            # and (slot1@row0, slot3@row64) target different PSUM banks, so
            # each pair streams in ~256 shared cycles instead of 2x256.
            xt3 = tiles[2]
            for s01 in (0, 1):
                for pb, slot in ((0, s01), (64, s01 + 2)):
                    nc.tensor.matmul(
                        pg[:, slot * CH:slot * CH + CH],
                        wih3_sb[pb:pb + K3R, slot * 128:(slot + 1) * 128],
                        xt3[pb:pb + K3R, c0_:c0_ + CH],
                        start=False, stop=False)

        # heads: wcat stationary (16-col LDW), hs moving; out [16, NG*CH]
        # transposed (host untransposes). 2 chunks share one PSUM bank.
        ph_tiles = {}

        def emit_heads(g):
            pair, loc = g // 2, g % 2
            if loc == 0:
                ph_tiles[pair] = php.tile([16, 512], f32, tag="ph", name="ph")
            ph = ph_tiles[pair]
            nc.tensor.matmul(ph[:, loc * CH:loc * CH + CH],
                             wcat_sb[:, :], hs_all[:, g * CH:g * CH + CH],
                             start=(loc == 0), stop=(loc == 1 or g == NG - 1))

        def flush_heads(pair):
            # fused bias-add + PSUM->SBUF copy, one DMA per chunk pair
            ph = ph_tiles.pop(pair)
            n = min(512, NG * CH - pair * 512)
            ob = wk.tile([16, 512], f32, tag="ob", name="ob")
            nc.vector.scalar_tensor_tensor(
                ob[:, 0:n], ph[:, 0:n], 1.0, bhd_sb[:, 0:n], OP.mult, OP.add)
            nc.scalar.dma_start(out=out_d[:, pair * 512:pair * 512 + n],
                                in_=ob[:, 0:n])

        emit_xw(0)
        emit_xw(1)

        hm_ref = {}
        c_ref = {}
        for j in range(NJ):
            hm_ref[j - NJ] = h0_sb[:, j * CH:(j + 1) * CH]
            c_ref[j - NJ] = c0_sb[:, j * CH:(j + 1) * CH]

        sig_ref = {}
        th_ref = {}

        # ---- the recurrence over global chunks ----
        for g in range(NG):
            if g > 0 and g - 1 in coff:
                load_slab(coff.index(g - 1) + 2)
            mt = mts[g_slab[g]]
            mc = (g - coff[g_slab[g]]) * CH

            # PE: W_hh accumulation for chunk g (chain input from g-3)
            pg = psum_tiles.pop(g)
            hm_in = hm_ref.pop(g - NJ)
            for slot in range(4):
                nc.tensor.matmul(
                    pg[:, slot * CH:slot * CH + CH],
                    whh_sb[:, slot * 128:(slot + 1) * 128],
                    hm_in, start=False, stop=(slot in (1, 3)))

            # PE fillers: xW two chunks ahead, heads two chunks behind
            emit_xw(g + 2)
            if g < 9:
                # keep PE busy/warm through pipeline priming
                fillers(6 if g == 0 else 4 if g < 4 else 2)
            if g >= 2:
                emit_heads(g - 2)

            # ACT: tanh of last chunk's c (input was ready long ago), then
            # the one big sigmoid for this chunk.
            if g >= 1:
                th = wk.tile([128, CH], bf16, tag="th", name="th")
                nc.scalar.activation(th[:, :], c_ref[g - 1][:, :], AF.Tanh)
                th_ref[g - 1] = th
            sig = wk.tile([128, 1024], bf16, tag="sig", name="sig")
            nc.scalar.activation(sig[:, :], pg[:, :], AF.Sigmoid)
            sig_ref[g] = sig

            # DVE: finish chunk g-1 (hs into history, masked hm for the
            # chain), then chunk g's c-path.
            if g >= 1:
                sp = sig_ref[g - 1]
                col = (g - 1) * CH
                nc.vector.tensor_mul(hs_all[:, col:col + CH],
                                     sp[:, 0:CH], th_ref.pop(g - 1)[:, :])
                if (g - 1) // NJ < L - 1:
                    hm = wk.tile([128, CH], bf16, tag="hm", name="hm")
                    nc.vector.tensor_mul(hm[:, :], hs_all[:, col:col + CH],
                                         mts[g_slab[g - 1]][
                                             :, (g - 1 - coff[g_slab[g - 1]])
                                             * CH:(g - coff[g_slab[g - 1]]) * CH])
                    hm_ref[g - 1] = hm
                del sig_ref[g - 1]
            t2 = wk.tile([128, CH], bf16, tag="t2", name="t2")
            nc.vector.tensor_mul(t2[:, :], sig[:, 2 * CH:3 * CH],
                                 c_ref.pop(g - NJ)[:, :])
            u = wk.tile([128, CH], bf16, tag="u", name="u")
            nc.vector.scalar_tensor_tensor(
                u[:, :], sig[:, 3 * CH:4 * CH], 0.5, sig[:, CH:2 * CH],
                OP.subtract, OP.mult)
            cn = wk.tile([128, CH], bf16, tag="cn", name="cn")
            nc.vector.scalar_tensor_tensor(
                cn[:, :], u[:, :], 2.0, t2[:, :], OP.mult, OP.add)
            c_ref[g] = cn
            if g >= 2 and (g - 2) % 2 == 1:
                flush_heads((g - 2) // 2)

        # ---- drain: last chunk's tanh/hs, remaining heads ----
        th = wk.tile([128, CH], bf16, tag="th", name="th")
        nc.scalar.activation(th[:, :], c_ref[NG - 1][:, :], AF.Tanh)
        nc.vector.tensor_mul(hs_all[:, (NG - 1) * CH:NG * CH],
                             sig_ref[NG - 1][:, 0:CH], th[:, :])
        for g in (NG - 2, NG - 1):
            emit_heads(g)
            if g % 2 == 1 or g == NG - 1:
                flush_heads(g // 2)

    nc.compile()
    return nc


_NC = {}


def _get_nc(L):
    if L not in _NC:
        _NC[L] = build_nc(L)
    return _NC[L]


def _segments(done):
    """Split every env's timeline at done=1 into segments, deal segments
    across cores, pack each core's segments into WIDTH column chains.
    Returns (L, plan): L = max chain depth; plan[c] = per-core slot maps."""
    done = np.asarray(done, dtype=np.int32)
    segs = []  # (length, t0, b, initial)
    for b in range(B):
        col = done[:, b]
        starts = np.flatnonzero(col == 1)
        if len(starts) == 0 or starts[0] != 0:
            starts = np.r_[0, starts]
        lens = np.diff(np.r_[starts, T])
        for t0, ln in zip(starts.tolist(), lens.tolist()):
            segs.append((int(ln), int(t0), b, t0 == 0 and col[0] == 0))

    segs.sort(key=lambda s: (-s[0], s[1], s[2]))
    init_segs = [s for s in segs if s[3]]
    rest_segs = [s for s in segs if not s[3]]

    # deal across cores: initial segments round-robin, rest LPT by positions
    core_segs = [[] for _ in range(NCORES)]
    core_load = [0] * NCORES
    for i, s in enumerate(init_segs):
        c = i % NCORES
        core_segs[c].append(s)
        core_load[c] += s[0]
    heap = [(core_load[c], c) for c in range(NCORES)]
    heapq.heapify(heap)
    for s in rest_segs:
        load, c = heapq.heappop(heap)
        core_segs[c].append(s)
        heapq.heappush(heap, (load + s[0], c))

    # pack each core's segments into WIDTH columns (initial segs first, one
    # per column at position 0; then LPT over all columns)
    plan = []
    Lmax = 0
    for c in range(NCORES):
        ini = [s for s in core_segs[c] if s[3]]
        oth = [s for s in core_segs[c] if not s[3]]
        oth.sort(key=lambda s: (-s[0], s[1], s[2]))
        cols = [[] for _ in range(WIDTH)]
        fill = [0] * WIDTH
        for i, s in enumerate(ini):
            cols[i].append(s)
            fill[i] = s[0]
        heap = [(fill[w], w) for w in range(WIDTH)]
        heapq.heapify(heap)
        for s in oth:
            f, w = heapq.heappop(heap)
            cols[w].append(s)
            heapq.heappush(heap, (f + s[0], w))
        Lc = max(sum(s[0] for s in cols[w]) for w in range(WIDTH))
        Lmax = max(Lmax, Lc)
        plan.append({"cols": cols, "n_init": len(ini)})
    L = Lmax

    # build slot maps
    for c in range(NCORES):
        cols = plan[c]["cols"]
        src = np.full((L, WIDTH), -1, dtype=np.int64)
        de = np.ones((L, WIDTH), dtype=np.float32)  # done row (pads -> 1)
        m = np.zeros((L, WIDTH), dtype=np.float32)
        h0b = np.full(WIDTH, -1, dtype=np.int64)  # env idx for init state
        for w in range(WIDTH):
            s_off = 0
            for (ln, t0, b, initial) in cols[w]:
                ts = np.arange(t0, t0 + ln)
                src[s_off:s_off + ln, w] = ts * B + b
                de[s_off, w] = 0.0 if initial else 1.0
                de[s_off + 1:s_off + ln, w] = 0.0
                m[s_off:s_off + ln - 1, w] = 1.0
                if initial:
                    h0b[w] = b
                s_off += ln
        plan[c] = {"src": src, "de": de, "m": m, "h0b": h0b}
    return L, plan


def _make_in_maps(inputs, L, plan):
    import ml_dtypes

    bf16 = ml_dtypes.bfloat16
    x = np.asarray(inputs["x"], dtype=np.float32)
    done = np.asarray(inputs["done"], dtype=np.int32)
    h0 = np.asarray(inputs["h0"], dtype=np.float32).reshape(B, H)
    c0 = np.asarray(inputs["c0"], dtype=np.float32).reshape(B, H)
    Wih = np.asarray(inputs["W_ih"], dtype=np.float32)
    Whh = np.asarray(inputs["W_hh"], dtype=np.float32)
    bias = (np.asarray(inputs["b_ih"], dtype=np.float32)
            + np.asarray(inputs["b_hh"], dtype=np.float32)).reshape(4 * H)
    Wpi = np.asarray(inputs["W_pi"], dtype=np.float32)
    bpi = np.asarray(inputs["b_pi"], dtype=np.float32).reshape(A)
    Wv = np.asarray(inputs["W_v"], dtype=np.float32)
    bv = np.asarray(inputs["b_v"], dtype=np.float32).reshape(1)

    # gate order i,f,g,o -> o,i,f,g; g block (weights + bias) pre-doubled
    order = np.r_[384:512, 0:128, 128:256, 256:384]
    GS = 384  # g block offset after reorder
    FS = 256  # f block offset after reorder
    WihR = Wih[order].copy()
    WihR[GS:GS + 128] *= 2.0
    WhhR = Whh[order].copy()
    WhhR[GS:GS + 128] *= 2.0
    biasR = bias[order].copy()
    biasR[GS:GS + 128] *= 2.0

    wih_aug = np.zeros((INA, 512), dtype=np.float32)
    wih_aug[0:IN] = WihR.T
    wih_aug[IN] = biasR
    wih_aug[IN + 1, FS:FS + 128] = -30.0  # done kills the f gate
    wih_bf = wih_aug.astype(bf16)
    wih3 = np.zeros((102, 512), dtype=np.float32)
    wih3[0:K3R] = wih_aug[K3:INA]  # duplicated at partition bases 0 and 64
    wih3[64:64 + K3R] = wih_aug[K3:INA]
    wih3_bf = wih3.astype(bf16)
    whh_bf = np.ascontiguousarray(WhhR.T).astype(bf16)

    wcat = np.zeros((128, 16), dtype=np.float32)
    wcat[:, 0:A] = Wpi.T
    wcat[:, A] = Wv[0]
    wcat_bf = wcat.astype(bf16)
    bgrp = np.zeros(16, dtype=np.float32)
    bgrp[0:A] = bpi
    bgrp[A] = bv[0]
    bhd = np.ascontiguousarray(
        np.broadcast_to(bgrp[:, None], (16, 512)))  # [16, 512]

    xT = np.ascontiguousarray(x.transpose(2, 0, 1).reshape(IN, T * B))
    h0T = h0.T  # [H, B]
    c0T = c0.T

    in_maps = []
    for c in range(NCORES):
        p = plan[c]
        src = p["src"].reshape(-1)  # [L*WIDTH] in slot order
        valid = src >= 0
        xt = np.zeros((INA, L * WIDTH), dtype=np.float32)
        xt[0:IN, valid] = xT[:, src[valid]]
        xt[IN] = 1.0
        xt[IN + 1] = p["de"].reshape(-1)

        m_bc = np.ascontiguousarray(np.broadcast_to(
            p["m"].reshape(1, L * WIDTH), (128, L * WIDTH))).astype(bf16)

        h0c = np.zeros((H, WIDTH), dtype=np.float32)
        c0c = np.zeros((H, WIDTH), dtype=np.float32)
        wsel = p["h0b"] >= 0
        h0c[:, wsel] = h0T[:, p["h0b"][wsel]]
        c0c[:, wsel] = c0T[:, p["h0b"][wsel]]

        in_maps.append({
            "xt": xt.astype(bf16),
            "m": m_bc,
            "h0": h0c.astype(bf16),
            "c0": c0c.astype(bf16),
            "wih": wih_bf,
            "wih3": wih3_bf,
            "whh": whh_bf,
            "wcat": wcat_bf,
            "bhd": bhd,
        })
    return in_maps


def _try_device_reset():
    try:
        import ctypes

        import jax

        jax.devices()
        lib = ctypes.CDLL("/opt/axon/libaxon_pjrt.so")
        if hasattr(lib, "axon_reset"):
            lib.axon_reset.restype = ctypes.c_int64
            lib.axon_reset()
    except Exception:
        pass


def kernel(**inputs):
    from concourse.bass_utils import run_bass_kernel_spmd

    done = np.asarray(inputs["done"], dtype=np.int32)
    L, plan = _segments(done)
    nc = _get_nc(L)
    in_maps = _make_in_maps(inputs, L, plan)
    try:
        res = run_bass_kernel_spmd(nc, in_maps, core_ids=list(range(NCORES)))
    except Exception:
        _try_device_reset()
        res = run_bass_kernel_spmd(nc, in_maps, core_ids=list(range(NCORES)))
    full = np.empty((T * B, NOUT), dtype=np.float32)
    for c in range(NCORES):
        out = np.ascontiguousarray(res.results[c]["out"].T)  # [NG*CH, 16]
        src = plan[c]["src"].reshape(-1)
        valid = src >= 0
        full[src[valid]] = out[valid][:, 0:NOUT]
    return full
